# revision 19
# baseline (speedup 1.0000x reference)
"""Trainium2 Bass kernel for nn_AttentionModel_87462714015827.

3-layer transformer encoder: B=16, S=1024, D=128, H=8 heads (DH=16),
FFN hidden 512, final 6-class projection.

Sharding: data-parallel over batch across 8 NeuronCores (2 batches/core),
all parameters replicated, no collectives.

Architecture of this version (v2):
  - ScalarE (ACT) does NOTHING but softmax exp; it is the kernel's
    metronome (~1.11us per 2-PSUM-bank exp, ~143us/layer).
  - All matmul streams are bf16 (f32 PSUM accumulate): weights are
    DMA'd f32 and cast on-chip in the prologue.
  - NO PE transposes: every layout flip (x^T, x1^T, o, f) is a hardware
    DMA transpose (XBAR) issued on the idle Sync engine, bf16.
  - PSUM: score pool 3 bufs x 2 banks (so the 4 score matmuls of a kt
    issue back-to-back and run 4-way row-tiled concurrent), o 1 bank,
    mp 1 bank for all projection/FFN matmuls.
  - The LN/FFN/projection pipeline for a 512-token group is emitted as
    a queue of small closures ("fillers"), popped one per kt iteration
    of the attention loop, so FFN/proj PE+DVE work flows inside the
    PE/DVE slack under the continuous ACT exp stream.
  - Scores use K=32 (each head duplicated in the slab) -> 2x score,
    folded into the exp scale.
"""

import os
import sys
from collections import deque

import numpy as np

for _p in ("/opt/trn_rl_repo", "/root/.axon_site/_ro/trn_rl_repo"):
    if os.path.isdir(_p) and _p not in sys.path:
        sys.path.insert(0, _p)

B, S, D, H, L = 16, 1024, 128, 8, 3
DFF = 4 * D          # 512
DH = D // H          # 16
NCLS = 6
NCORES = 8
B_LOC = B // NCORES  # 2
TOK = B_LOC * S      # 2048
TT = TOK // 128      # 16 token tiles per core
TPB = S // 128       # 8 token tiles per batch
P = 128
NQUAD = 2            # head quads (4 heads each)
KT = TPB             # 8 k tiles of 128 per batch
QCW = 512            # q-chunk width
NSUB = QCW // P      # 4 token tiles per group
NG = 4               # token groups (b, qc) per layer

_CACHE = {}


def _build_nc():
    import concourse.bass as bass
    import concourse.mybir as mybir
    import concourse.tile as tile
    from concourse import bacc

    dt = mybir.dt
    f32 = dt.float32
    f32r = dt.float32r
    bf16 = dt.bfloat16
    i32 = dt.int32
    AF = mybir.ActivationFunctionType
    OP = mybir.AluOpType

    nc = bacc.Bacc("TRN2", target_bir_lowering=False)

    # ---- DRAM I/O ----
    x_d = nc.dram_tensor("x", [B_LOC, S, D], f32, kind="ExternalInput")
    wq_d = nc.dram_tensor("Wq", [L, D, D], f32, kind="ExternalInput")
    bq_d = nc.dram_tensor("bq", [L, D], f32, kind="ExternalInput")
    wk_d = nc.dram_tensor("Wk", [L, D, D], f32, kind="ExternalInput")
    bk_d = nc.dram_tensor("bk", [L, D], f32, kind="ExternalInput")
    wv_d = nc.dram_tensor("Wv", [L, D, D], f32, kind="ExternalInput")
    bv_d = nc.dram_tensor("bv", [L, D], f32, kind="ExternalInput")
    l1g_d = nc.dram_tensor("ln1_g", [L, D], f32, kind="ExternalInput")
    l1b_d = nc.dram_tensor("ln1_b", [L, D], f32, kind="ExternalInput")
    w1_d = nc.dram_tensor("W1", [L, D, DFF], f32, kind="ExternalInput")
    b1_d = nc.dram_tensor("b1", [L, DFF], f32, kind="ExternalInput")
    w2_d = nc.dram_tensor("W2", [L, DFF, D], f32, kind="ExternalInput")
    b2_d = nc.dram_tensor("b2", [L, D], f32, kind="ExternalInput")
    l2g_d = nc.dram_tensor("ln2_g", [L, D], f32, kind="ExternalInput")
    l2b_d = nc.dram_tensor("ln2_b", [L, D], f32, kind="ExternalInput")
    wout_d = nc.dram_tensor("Wout", [D, NCLS], f32, kind="ExternalInput")
    bout_d = nc.dram_tensor("bout", [NCLS], f32, kind="ExternalInput")
    out_d = nc.dram_tensor("out", [B_LOC, S, NCLS], f32, kind="ExternalOutput")

    with tile.TileContext(nc) as tc:
        from contextlib import ExitStack

        ctx = ExitStack()
        cpool = ctx.enter_context(tc.tile_pool(name="const", bufs=1))
        apool = ctx.enter_context(tc.tile_pool(name="acts", bufs=2))
        gpool = ctx.enter_context(tc.tile_pool(name="grp", bufs=2))
        xpool = ctx.enter_context(tc.tile_pool(name="xprev", bufs=6))
        epool = ctx.enter_context(tc.tile_pool(name="epool", bufs=6))
        small = ctx.enter_context(tc.tile_pool(name="small", bufs=2))
        ps_sc = ctx.enter_context(tc.tile_pool(name="ps_sc", bufs=3, space="PSUM"))
        ps_o = ctx.enter_context(tc.tile_pool(name="ps_o", bufs=1, space="PSUM"))
        ps_mp = ctx.enter_context(tc.tile_pool(name="ps_mp", bufs=1, space="PSUM"))

        # ================= prologue: loads =================
        # x first: it gates the longest dependency chain.
        x_sb = cpool.tile([P, TT, D], f32)
        nc.gpsimd.dma_start(out=x_sb, in_=x_d.rearrange("b (t p) d -> p (b t) d", p=P))

        # plain (few-descriptor) f32 staging loads of all weights
        wq_st = cpool.tile([P, L, D], f32)
        nc.gpsimd.dma_start(out=wq_st, in_=wq_d.rearrange("l d e -> d l e"))
        wk_st = cpool.tile([P, L, D], f32)
        nc.gpsimd.dma_start(out=wk_st, in_=wk_d.rearrange("l d e -> d l e"))
        wv_st = cpool.tile([P, L, D], f32)
        nc.gpsimd.dma_start(out=wv_st, in_=wv_d.rearrange("l d e -> d l e"))
        w1_st = cpool.tile([P, L, DFF], f32)
        nc.gpsimd.dma_start(out=w1_st, in_=w1_d.rearrange("l d f -> d l f"))
        w2_st = cpool.tile([P, L, 4, D], f32)
        nc.gpsimd.dma_start(out=w2_st, in_=w2_d.rearrange("l (c p) e -> p l c e", p=P))

        # Q/K bias slabs in slab partition order via partition-scatter DMA:
        # partition 32j+16u+dh holds bq[l, 64g+16j+dh] (u duplicates).
        def bias_slab(src_d):
            t = cpool.tile([P, L, NQUAD], f32, name=f"bslab{src_d.name}")
            for g in range(NQUAD):
                for u in range(2):
                    src = bass.AP(
                        tensor=src_d[0, 0].tensor, offset=64 * g,
                        ap=[[16, 4], [1, DH], [D, L]],
                    )
                    nc.gpsimd.dma_start(
                        out=t[:, :, g].rearrange("(j u e) l -> j u e l",
                                                 j=4, u=2)[:, u, :, :],
                        in_=src,
                    )
            return t

        bq_sb = bias_slab(bq_d)
        bk_sb = bias_slab(bk_d)

        # partition-replicated per-feature vectors
        _repn = [0]

        def rep_load(src_ap, shape):
            _repn[0] += 1
            t = cpool.tile([P] + shape, f32, name=f"rep{_repn[0]}")
            bc = bass.AP(tensor=src_ap.tensor, offset=src_ap.offset,
                         ap=[[0, P]] + [list(e) for e in src_ap.ap])
            nc.gpsimd.dma_start(out=t, in_=bc)
            return t

        bv_rep = rep_load(bv_d[:, :], [L, D])
        l1b_rep = rep_load(l1b_d[:, :], [L, D])
        l1g_rep = rep_load(l1g_d[:, :], [L, D])
        l2g_rep = rep_load(l2g_d[:, :], [L, D])
        l2b_rep = rep_load(l2b_d[:, :], [L, D])
        bout_rep = rep_load(bout_d[:], [NCLS])

        # b2 in feature-major (per-partition) form, applied at the f^T drain
        b2_col = cpool.tile([P, L], f32)
        nc.gpsimd.dma_start(out=b2_col, in_=b2_d.rearrange("l d -> d l"))
        wout_st = cpool.tile([P, NCLS], f32)
        nc.gpsimd.dma_start(out=wout_st, in_=wout_d[:, :])

        # ================= prologue: on-chip casts to bf16 =================
        # Q/K weight slabs: quad g, head 4g+j at cols 32j..32j+15 AND
        # duplicated at 32j+16..31 (K=32 scores read both copies).
        wq_sb = cpool.tile([P, L, NQUAD, P], bf16)
        wk_sb = cpool.tile([P, L, NQUAD, P], bf16)
        for (w_st, w_sb) in ((wq_st, wq_sb), (wk_st, wk_sb)):
            for l in range(L):
                for g in range(NQUAD):
                    src = (w_st[:, l, 64 * g : 64 * g + 64]
                           .rearrange("d (j e) -> d j e", j=4)
                           [:, :, None, :].to_broadcast([P, 4, 2, DH]))
                    nc.vector.tensor_copy(
                        w_sb[:, l, g, :].rearrange("p (j u e) -> p j u e",
                                                   j=4, u=2),
                        src,
                    )
        wv_sb = cpool.tile([P, L, D], bf16)
        nc.gpsimd.tensor_copy(wv_sb, wv_st)
        w1_sb = cpool.tile([P, L, DFF], bf16)
        nc.gpsimd.tensor_copy(w1_sb, w1_st)
        w2_sb = cpool.tile([P, L, 4, D], bf16)
        nc.gpsimd.tensor_copy(w2_sb, w2_st)
        wout_sb = cpool.tile([P, NCLS], bf16)
        nc.vector.tensor_copy(wout_sb, wout_st)

        def r(ap):
            return ap if ap.dtype == f32r else ap.bitcast(f32r)

        # HAM warmup: dense matmuls so the PE clock-gate opens before the
        # projection work starts (overlaps the weight DMAs above).
        wu16 = cpool.tile([P, 512], bf16)
        nc.vector.memset(wu16, 1.0)
        wup = ps_mp.tile([P, 512], f32, tag="mp", name="wup")
        for w in range(8):
            nc.tensor.matmul(wup, wu16[:, 0:P], wu16, start=True, stop=True)

        # x in bf16, then x^T via one DMA transpose
        x16 = cpool.tile([P, TT, D], bf16)
        nc.vector.tensor_copy(x16, x_sb)

        SC = 1.0 / np.sqrt(np.float32(DH))

        # ---------- helpers ----------
        def dma_T(out2, in2):
            """XBAR transpose of one [128,128] bf16 tile on the Sync HWDGE:
            out[do, m] = in[m, do]."""
            nc.sync.dma_start(out=out2, in_=in2, transpose=True)

        def rsqrt_dve(rstd, var_ap, eps, w, tagp):
            """rstd = 1/sqrt(var+eps) on DVE (magic seed + 3 Newton steps)."""
            ve = small.tile([P, w], f32, tag="ve", name=f"ve{tagp}")
            nc.vector.tensor_scalar(out=ve, in0=var_ap, scalar1=float(eps),
                                    scalar2=None, op0=OP.add)
            yi = rstd.bitcast(i32)
            nc.vector.tensor_scalar(out=yi, in0=ve.bitcast(i32), scalar1=1,
                                    scalar2=None, op0=OP.logical_shift_right)
            nc.vector.tensor_scalar(out=yi, in0=yi, scalar1=0x5F3759DF,
                                    scalar2=-1, op0=OP.subtract, op1=OP.mult)
            nt = small.tile([P, w], f32, tag="nt", name=f"nt{tagp}")
            for _ in range(3):
                nc.vector.tensor_tensor(nt, rstd, rstd, OP.mult)
                nc.vector.tensor_tensor(nt, nt, ve, OP.mult)
                nc.vector.tensor_scalar(out=nt, in0=nt, scalar1=-0.5,
                                        scalar2=1.5, op0=OP.mult, op1=OP.add)
                nc.vector.tensor_tensor(rstd, rstd, nt, OP.mult)

        # per-layer slabs (apool bufs=2 alternates even/odd layers)
        def alloc_layer_slabs(l):
            sl = {}
            sl["qt"] = apool.tile([P, NQUAD, TOK], bf16, tag="qt", name=f"qt{l}")
            sl["kt"] = apool.tile([P, NQUAD, TOK], bf16, tag="kt", name=f"kt{l}")
            sl["v"] = apool.tile([P, TT, H, 32], bf16, tag="v", name=f"v{l}")
            nc.vector.memset(sl["v"][:, :, :, DH], 1.0)
            nc.vector.memset(sl["v"][:, :, :, DH + 1 : 32], 0.0)
            return sl

        def emit_qk_proj_pair(l, sl, xt_sl, proj, g, gq0):
            """One 512-token QK projection matmul + relu drain into the
            qt/kt slab of layer l. gq0: global token offset (512-aligned)."""
            w_sb, b_sb, dston = (
                (wq_sb, bq_sb, "qt") if proj == 0 else (wk_sb, bk_sb, "kt"))
            pp = ps_mp.tile([P, 512], f32, tag="mp", name=f"pj{l}{proj}{g}{gq0}")
            nc.tensor.matmul(pp, w_sb[:, l, g, :], xt_sl, start=True, stop=True)
            nc.vector.tensor_scalar(
                out=sl[dston][:, g, gq0 : gq0 + 512], in0=pp,
                scalar1=b_sb[:, l, g : g + 1], scalar2=0.0,
                op0=OP.add, op1=OP.max,
            )

        def emit_v_proj(l, sl, xt, t):
            """V projection for one 128-token tile + bias/relu drain."""
            pv = ps_mp.tile([P, D], f32, tag="mp", name=f"pv{l}{t}")
            nc.tensor.matmul(pv, xt[:, t * P : (t + 1) * P], wv_sb[:, l, :],
                             start=True, stop=True)
            nc.vector.tensor_tensor(
                sl["v"][:, t, :, 0:DH],
                pv.rearrange("p (h e) -> p h e", h=H),
                bv_rep[:, l, :].rearrange("p (h e) -> p h e", h=H),
                OP.add,
            )
            nc.vector.tensor_scalar(
                out=sl["v"][:, t, :, 0:DH], in0=sl["v"][:, t, :, 0:DH],
                scalar1=0.0, scalar2=None, op0=OP.max,
            )

        # ================= group pipeline stages =================
        # Token group G=(b,qc): 4 token tiles t0=b*TPB+qc*NSUB .. +3,
        # global q offset gq0 = G*512 in (b t)-major token order.

        def stage1(l, G, og, xprev_src, gst):
            """res1 = o + xprev; LN1 -> xn; t1 = xn*g1+b1 (+bf16); x1t DMA."""
            b, qc = divmod(G, 2)
            t0 = b * TPB + qc * NSUB
            res = gpool.tile([P, NSUB, D], f32, tag="res", name=f"res{l}{G}")
            mv = small.tile([P, NSUB, 2], f32, tag="mv", name=f"mv1{l}{G}")
            rstd = small.tile([P, NSUB], f32, tag="rstd", name=f"rs1{l}{G}")
            for i in range(NSUB):
                nc.vector.tensor_tensor(
                    res[:, i, :], og[:, i, :], xprev_src(t0 + i), OP.add)
                st6 = small.tile([P, 6], f32, tag="st6", name=f"s1{l}{G}{i}")
                nc.vector.bn_stats(out=st6, in_=res[:, i, :])
                nc.vector.bn_aggr(out=mv[:, i, :], in_=st6)
            rsqrt_dve(rstd, mv[:, :, 1], 1e-8, NSUB, f"a{l}{G}")
            xn = gpool.tile([P, NSUB, D], f32, tag="xn", name=f"xn{l}{G}")
            for i in range(NSUB):
                nc.vector.tensor_scalar(
                    out=xn[:, i, :], in0=res[:, i, :],
                    scalar1=mv[:, i, 0:1], scalar2=rstd[:, i : i + 1],
                    op0=OP.subtract, op1=OP.mult,
                )
            t1 = gpool.tile([P, NSUB, D], f32, tag="t1", name=f"t1{l}{G}")
            t116 = gpool.tile([P, NSUB, D], bf16, tag="t116", name=f"t116{l}{G}")
            for i in range(NSUB):
                nc.gpsimd.tensor_tensor(
                    t1[:, i, :], xn[:, i, :], l1g_rep[:, l, :], OP.mult)
                nc.gpsimd.tensor_tensor(
                    t1[:, i, :], t1[:, i, :], l1b_rep[:, l, :], OP.add)
            nc.gpsimd.tensor_copy(t116, t1)
            x1t = gpool.tile([P, NSUB, P], bf16, tag="x1t", name=f"x1t{l}{G}")
            for i in range(NSUB):
                dma_T(x1t[:, i, :], t116[:, i, :])
            gst.update(t1=t1, x1t=x1t)

        def stage2_ffn1(l, G, gst, c):
            """FFN1 block c: matmul over x1t + relu drain into ht."""
            if c == 0:
                gst["ht"] = gpool.tile([P, 4, 512], bf16, tag="ht",
                                       name=f"ht{l}{G}")
            x1t = gst["x1t"]
            pp = ps_mp.tile([P, 512], f32, tag="mp", name=f"ph{l}{G}{c}")
            nc.tensor.matmul(
                pp, w1_sb[:, l, c * P : (c + 1) * P],
                x1t.rearrange("d s m -> d (s m)"), start=True, stop=True)
            nc.vector.tensor_scalar(
                out=gst["ht"][:, c, :], in0=pp,
                scalar1=b1c_sb[:, l, c : c + 1], scalar2=0.0,
                op0=OP.add, op1=OP.max,
            )

        def stage2_ffn2(l, G, gst):
            """FFN2: 4 accumulating matmuls + b2 drain (bf16) + f DMA-T."""
            pf = ps_mp.tile([P, 512], f32, tag="mp", name=f"pf{l}{G}")
            for c in range(4):
                nc.tensor.matmul(pf, w2_sb[:, l, c, :], gst["ht"][:, c, :],
                                 start=(c == 0), stop=(c == 3))
            ft16 = gpool.tile([P, NSUB, P], bf16, tag="ft16", name=f"ft{l}{G}")
            nc.vector.tensor_scalar(
                out=ft16.rearrange("d s m -> d (s m)"), in0=pf,
                scalar1=b2_col[:, l : l + 1], scalar2=None, op0=OP.add)
            ftt = gpool.tile([P, NSUB, P], bf16, tag="ftt", name=f"ftt{l}{G}")
            for i in range(NSUB):
                dma_T(ftt[:, i, :], ft16[:, i, :])
            gst["ftt"] = ftt

        def stage3(l, G, gst, xprev_next, xt_next):
            """res2 = f + t1; LN2 -> xn2; xprev(l+1); x^T(l+1) DMA."""
            res2 = gpool.tile([P, NSUB, D], f32, tag="res2", name=f"re2{l}{G}")
            mv = small.tile([P, NSUB, 2], f32, tag="mv", name=f"mv2{l}{G}")
            rstd = small.tile([P, NSUB], f32, tag="rstd", name=f"rs2{l}{G}")
            for i in range(NSUB):
                nc.vector.tensor_tensor(
                    res2[:, i, :], gst["ftt"][:, i, :], gst["t1"][:, i, :],
                    OP.add)
                st6 = small.tile([P, 6], f32, tag="st6", name=f"s2{l}{G}{i}")
                nc.vector.bn_stats(out=st6, in_=res2[:, i, :])
                nc.vector.bn_aggr(out=mv[:, i, :], in_=st6)
            rsqrt_dve(rstd, mv[:, :, 1], 1e-6, NSUB, f"b{l}{G}")
            xn2 = gpool.tile([P, NSUB, D], f32, tag="xn2", name=f"xn2{l}{G}")
            for i in range(NSUB):
                nc.vector.tensor_scalar(
                    out=xn2[:, i, :], in0=res2[:, i, :],
                    scalar1=mv[:, i, 0:1], scalar2=rstd[:, i : i + 1],
                    op0=OP.subtract, op1=OP.mult,
                )
            xp16 = gpool.tile([P, NSUB, D], bf16, tag="xp16", name=f"xp16{l}{G}")
            for i in range(NSUB):
                nc.gpsimd.tensor_tensor(
                    xprev_next[:, i, :], xn2[:, i, :], l2g_rep[:, l, :],
                    OP.mult)
                nc.gpsimd.tensor_tensor(
                    xprev_next[:, i, :], xprev_next[:, i, :], l2b_rep[:, l, :],
                    OP.add)
            nc.gpsimd.tensor_copy(xp16, xprev_next)
            gq0 = G * 512
            for i in range(NSUB):
                dma_T(xt_next[:, gq0 + i * P : gq0 + (i + 1) * P],
                      xp16[:, i, :])

        def emit_head(l, G, gst, xt, i):
            """Final 6-class projection for token tile i of group G."""
            b, qc = divmod(G, 2)
            t = b * TPB + qc * NSUB + i
            if i == 0:
                gst["o6"] = gpool.tile([P, NSUB, NCLS], f32, tag="o6",
                                       name=f"o6{G}")
            p6 = ps_mp.tile([P, NCLS], f32, tag="mp", name=f"p6{G}{i}")
            nc.tensor.matmul(p6, xt[:, t * P : (t + 1) * P], wout_sb,
                             start=True, stop=True)
            nc.vector.tensor_tensor(gst["o6"][:, i, :], p6, bout_rep, OP.add)
            if i == NSUB - 1:
                nc.sync.dma_start(
                    out=out_d.rearrange("b (t p) c -> p (b t) c", p=P)
                    [:, t - 3 : t + 1, :],
                    in_=gst["o6"],
                )

        # b1 column layout for FFN1 drains
        b1c_sb = cpool.tile([P, L, 4], f32)
        nc.gpsimd.dma_start(out=b1c_sb, in_=b1_d.rearrange("l (c p) -> p l c", p=P))

        # ================= layer-0 x^T + projections (serial prologue) ====
        xt0 = apool.tile([P, TOK], bf16, tag="xt", name="xt0")
        for t in range(TT):
            dma_T(xt0[:, t * P : (t + 1) * P], x16[:, t, :])
        slabs0 = alloc_layer_slabs(0)
        # QK proj in sc-pool pairs for double-buffered drains
        for proj in range(2):
            for g in range(NQUAD):
                for gq0 in range(0, TOK, 512):
                    emit_qk_proj_pair(0, slabs0, xt0[:, gq0 : gq0 + 512],
                                      proj, g, gq0)
        for t in range(TT):
            emit_v_proj(0, slabs0, xt0, t)

        # ================= main pipelined layer loop =================
        fillers = deque()

        def pump(n):
            for _ in range(min(n, len(fillers))):
                fillers.popleft()()

        xt_cur = xt0
        slabs_cur = slabs0
        # xprev tiles for layer1 groups are allocated lazily inside stage3
        xprev_next_tiles = {}
        gstate = {}

        for l in range(L):
            last = l == L - 1
            if not last:
                slabs_next = alloc_layer_slabs(l + 1)
                xt_next = apool.tile([P, TOK], bf16, tag="xt", name=f"xt{l+1}")
            else:
                slabs_next = None
                # final layer: xt_next holds (LN2*g2+b2)^T, the head input
                xt_next = apool.tile([P, TOK], bf16, tag="xt", name="xtF")

            og_tiles = {}
            for ci, (b, qc, g) in enumerate(
                    [(b, qc, g) for b in range(B_LOC) for qc in range(2)
                     for g in range(NQUAD)]):
                G = b * 2 + qc
                qs0 = b * S + qc * QCW
                if g == 0:
                    og_tiles[G] = gpool.tile([P, NSUB, D], f32, tag="og",
                                             name=f"og{l}{G}")
                o_ps = ps_o.tile([P, QCW], f32, tag="o", name=f"o{l}{b}{g}{qc}")
                prev_e = None
                for kt in range(KT):
                    ks0 = b * S + kt * P
                    # score QUAD (K=32: head duplicated -> 2x score)
                    scps = [ps_sc.tile([P, 2, QCW], f32, tag="sc",
                                       name=f"sc{l}{ci}{kt}{pr}")
                            for pr in range(2)]
                    for j in range(4):
                        nc.tensor.matmul(
                            scps[j // 2][:, j % 2, :],
                            slabs_cur["kt"][32 * j : 32 * j + 32, g,
                                            ks0 : ks0 + P],
                            slabs_cur["qt"][32 * j : 32 * j + 32, g,
                                            qs0 : qs0 + QCW],
                            start=True, stop=True,
                            tile_position=(32 * j, 0),
                        )
                    cur_e = []
                    for pr in range(2):
                        e_sb = epool.tile([P, 2, QCW], bf16, tag="e",
                                          name=f"e{l}{ci}{kt}{pr}")
                        nc.scalar.activation(
                            out=e_sb.rearrange("p a q -> p (a q)"),
                            in_=scps[pr].rearrange("p a q -> p (a q)"),
                            func=AF.Exp, scale=float(SC / 2),
                        )
                        cur_e.append(e_sb)
                    if prev_e is not None:
                        pkt, pe0, pe1 = prev_e
                        for j in range(4):
                            nc.tensor.matmul(
                                o_ps[32 * j : 32 * j + 32, :],
                                slabs_cur["v"][:, b * TPB + pkt, 4 * g + j, :],
                                (pe0 if j < 2 else pe1)[:, j % 2, :],
                                start=(pkt == 0), stop=False,
                                tile_position=(0, 32 * j),
                                skip_group_check=True,
                            )
                    prev_e = (kt, cur_e[0], cur_e[1])
                    pump(1)
                pkt, pe0, pe1 = prev_e
                for j in range(4):
                    nc.tensor.matmul(
                        o_ps[32 * j : 32 * j + 32, :],
                        slabs_cur["v"][:, b * TPB + pkt, 4 * g + j, :],
                        (pe0 if j < 2 else pe1)[:, j % 2, :],
                        start=False, stop=True,
                        tile_position=(0, 32 * j),
                        skip_group_check=True,
                    )
                # chunk epilogue: drain -> DMA transpose -> normalize
                ot16 = small.tile([P, QCW], bf16, tag="ot16",
                                  name=f"ot{l}{ci}")
                nc.vector.tensor_copy(ot16, o_ps)
                ott = gpool.tile([P, NSUB, P], bf16, tag="ott",
                                 name=f"ott{l}{ci}")
                for i in range(NSUB):
                    dma_T(ott[:, i, :], ot16[:, i * P : (i + 1) * P])
                rcp = small.tile([P, NSUB, 4], f32, tag="rcp",
                                 name=f"rcp{l}{ci}")
                nc.vector.reciprocal(rcp, ott[:, :, DH :: 32])
                og = og_tiles[G]
                nc.vector.tensor_tensor(
                    og[:, :, 64 * g : 64 * g + 64]
                        .rearrange("p s (j e) -> p s j e", j=4),
                    ott.rearrange("p s (j u) -> p s j u", j=4)[:, :, :, 0:DH],
                    rcp[:, :, :, None].to_broadcast([P, NSUB, 4, DH]),
                    OP.mult,
                )

                # after the 2nd quad of a group, enqueue its F/P1 pipeline
                if g == 1:
                    gst = gstate.setdefault((l, G), {})
                    og_t = og_tiles[G]
                    if l == 0:
                        xprev_src = lambda t: x_sb[:, t, :]
                    else:
                        xp = xprev_next_tiles[(l, G)]
                        xprev_src = lambda t, xp=xp, b=b, qc=qc: (
                            xp[:, t - b * TPB - qc * NSUB, :])
                    fillers.append(
                        lambda l=l, G=G, og_t=og_t, xs=xprev_src, gst=gst:
                        stage1(l, G, og_t, xs, gst))
                    for c in range(4):
                        fillers.append(
                            lambda l=l, G=G, gst=gst, c=c:
                            stage2_ffn1(l, G, gst, c))
                    fillers.append(
                        lambda l=l, G=G, gst=gst: stage2_ffn2(l, G, gst))
                    if not last:
                        xp_next = xpool.tile([P, NSUB, D], f32, tag="xp",
                                             name=f"xp{l+1}{G}")
                        xprev_next_tiles[(l + 1, G)] = xp_next
                        fillers.append(
                            lambda l=l, G=G, gst=gst, xp=xp_next, xt=xt_next:
                            stage3(l, G, gst, xp, xt))
                        sl_n = slabs_next
                        xt_n = xt_next
                        for proj in range(2):
                            for gg in range(NQUAD):
                                fillers.append(
                                    lambda l=l, G=G, sl=sl_n, xt=xt_n,
                                           proj=proj, gg=gg:
                                    emit_qk_proj_pair(
                                        l + 1, sl, xt[:, G * 512 : G * 512 + 512],
                                        proj, gg, G * 512))
                        b_, qc_ = divmod(G, 2)
                        t0 = b_ * TPB + qc_ * NSUB
                        for i in range(NSUB):
                            fillers.append(
                                lambda l=l, sl=sl_n, xt=xt_n, t=t0 + i:
                                emit_v_proj(l + 1, sl, xt, t))
                    else:
                        # final layer: stage3 computes (LN2*g2+b2) and its
                        # transpose xt_next; then the 6-class head.
                        xp_fin = xpool.tile([P, NSUB, D], f32, tag="xp",
                                            name=f"xpF{G}")
                        fillers.append(
                            lambda l=l, G=G, gst=gst, xp=xp_fin, xt=xt_next:
                            stage3(l, G, gst, xp, xt))
                        for i in range(NSUB):
                            fillers.append(
                                lambda l=l, G=G, gst=gst, xt=xt_next, i=i:
                                emit_head(l, G, gst, xt, i))

            xt_cur = xt_next
            slabs_cur = slabs_next

        pump(len(fillers))
        ctx.close()

    nc.compile()
    return nc


def _get_nc():
    if "nc" not in _CACHE:
        _CACHE["nc"] = _build_nc()
    return _CACHE["nc"]


def kernel(**inputs) -> np.ndarray:
    from concourse.bass_utils import run_bass_kernel_spmd

    nc = _get_nc()
    ins = {k: np.ascontiguousarray(np.asarray(v)) for k, v in inputs.items()}
    in_maps = []
    for c in range(NCORES):
        m = dict(ins)
        m["x"] = np.ascontiguousarray(ins["x"][c * B_LOC : (c + 1) * B_LOC])
        in_maps.append(m)
    res = run_bass_kernel_spmd(nc, in_maps, list(range(NCORES)))
    out = np.concatenate([res.results[c]["out"] for c in range(NCORES)], axis=0)
    return out


# revision 20
# speedup vs baseline: 1.4099x; 1.4099x over previous
"""Trainium2 Bass kernel for nn_AttentionModel_87462714015827.

3-layer transformer encoder: B=16, S=1024, D=128, H=8 heads (DH=16),
FFN hidden 512, final 6-class projection.

Sharding: data-parallel over batch across 8 NeuronCores (2 batches/core),
all parameters replicated, no collectives.

v2 architecture:
  - ScalarE (ACT) does NOTHING but softmax exp; it is the kernel's
    metronome (~1.11us per 2-PSUM-bank exp, ~143us/layer).
  - All matmul streams are fp16 (f32 PSUM accumulate; fp16's 10-bit
    mantissa keeps the end-to-end error ~8x below bf16). Weights are
    DMA'd f32 and cast on-chip in the prologue.
  - NO PE transposes: every layout flip (x^T, x1^T, o, f) is a hardware
    XBAR DMA transpose issued on the otherwise-idle Sync engine.
  - PSUM: score pool 3 bufs x 2 banks (the 4 score matmuls of a kt
    issue back-to-back and run 4-way row-tiled concurrent), o 1 bank,
    mp 1 bank for all projection/FFN matmuls.
  - The LN/FFN/projection pipeline for each 512-token group is emitted
    as 16 small closures ("fillers"), popped one per kt iteration of
    the attention loop (64 slots = 4 groups x 16 per layer), so that
    work flows inside the PE/DVE slack under the continuous exp stream.
  - Scores use K=32 (each head duplicated in the slab) -> 2x score,
    folded into the exp scale.
"""

import os
import sys
from collections import deque

import numpy as np

for _p in ("/opt/trn_rl_repo", "/root/.axon_site/_ro/trn_rl_repo"):
    if os.path.isdir(_p) and _p not in sys.path:
        sys.path.insert(0, _p)

B, S, D, H, L = 16, 1024, 128, 8, 3
DFF = 4 * D          # 512
DH = D // H          # 16
NCLS = 6
NCORES = 8
B_LOC = B // NCORES  # 2
TOK = B_LOC * S      # 2048
TT = TOK // 128      # 16 token tiles per core
TPB = S // 128       # 8 token tiles per batch
P = 128
NQUAD = 2            # head quads (4 heads each)
KT = TPB             # 8 k tiles of 128 per batch
QCW = 512            # q-chunk width
NSUB = QCW // P      # 4 token tiles per group
NG = 4               # token groups (b, qc) per layer

_CACHE = {}


def _build_nc():
    import concourse.bass as bass
    import concourse.mybir as mybir
    import concourse.tile as tile
    from concourse import bacc

    dt = mybir.dt
    f32 = dt.float32
    h16 = dt.float16
    i32 = dt.int32
    AF = mybir.ActivationFunctionType
    OP = mybir.AluOpType

    nc = bacc.Bacc("TRN2", target_bir_lowering=False)

    # ---- DRAM I/O ----
    x_d = nc.dram_tensor("x", [B_LOC, S, D], f32, kind="ExternalInput")
    wq_d = nc.dram_tensor("Wq", [L, D, D], f32, kind="ExternalInput")
    bq_d = nc.dram_tensor("bq", [L, D], f32, kind="ExternalInput")
    wk_d = nc.dram_tensor("Wk", [L, D, D], f32, kind="ExternalInput")
    bk_d = nc.dram_tensor("bk", [L, D], f32, kind="ExternalInput")
    wv_d = nc.dram_tensor("Wv", [L, D, D], f32, kind="ExternalInput")
    bv_d = nc.dram_tensor("bv", [L, D], f32, kind="ExternalInput")
    l1g_d = nc.dram_tensor("ln1_g", [L, D], f32, kind="ExternalInput")
    l1b_d = nc.dram_tensor("ln1_b", [L, D], f32, kind="ExternalInput")
    w1_d = nc.dram_tensor("W1", [L, D, DFF], f32, kind="ExternalInput")
    b1_d = nc.dram_tensor("b1", [L, DFF], f32, kind="ExternalInput")
    w2_d = nc.dram_tensor("W2", [L, DFF, D], f32, kind="ExternalInput")
    b2_d = nc.dram_tensor("b2", [L, D], f32, kind="ExternalInput")
    l2g_d = nc.dram_tensor("ln2_g", [L, D], f32, kind="ExternalInput")
    l2b_d = nc.dram_tensor("ln2_b", [L, D], f32, kind="ExternalInput")
    wout_d = nc.dram_tensor("Wout", [D, NCLS], f32, kind="ExternalInput")
    bout_d = nc.dram_tensor("bout", [NCLS], f32, kind="ExternalInput")
    out_d = nc.dram_tensor("out", [B_LOC, S, NCLS], f32, kind="ExternalOutput")

    with tile.TileContext(nc) as tc:
        from contextlib import ExitStack

        ctx = ExitStack()
        cpool = ctx.enter_context(tc.tile_pool(name="const", bufs=1))
        apool = ctx.enter_context(tc.tile_pool(name="acts", bufs=2))
        gpool = ctx.enter_context(tc.tile_pool(name="grp", bufs=2))
        xpool = ctx.enter_context(tc.tile_pool(name="xprev", bufs=6))
        epool = ctx.enter_context(tc.tile_pool(name="epool", bufs=6))
        small = ctx.enter_context(tc.tile_pool(name="small", bufs=2))
        ps_sc = ctx.enter_context(tc.tile_pool(name="ps_sc", bufs=3, space="PSUM"))
        ps_o = ctx.enter_context(tc.tile_pool(name="ps_o", bufs=1, space="PSUM"))
        ps_mp = ctx.enter_context(tc.tile_pool(name="ps_mp", bufs=1, space="PSUM"))

        # HAM warmup first: no DMA deps, opens the PE clock gate at t=0.
        wu16 = cpool.tile([P, 512], h16)
        nc.vector.memset(wu16, 1.0)
        wup = ps_mp.tile([P, 512], f32, tag="mp", name="wup")
        for w in range(8):
            nc.tensor.matmul(wup, wu16[:, 0:P], wu16, start=True, stop=True)

        # ================= prologue: loads (big ones on Sync HWDGE) ======
        x_sb = cpool.tile([P, TT, D], f32)
        nc.sync.dma_start(out=x_sb, in_=x_d.rearrange("b (t p) d -> p (b t) d", p=P))

        wq_st = cpool.tile([P, L, D], f32)
        nc.sync.dma_start(out=wq_st, in_=wq_d.rearrange("l d e -> d l e"))
        wk_st = cpool.tile([P, L, D], f32)
        nc.sync.dma_start(out=wk_st, in_=wk_d.rearrange("l d e -> d l e"))
        wv_st = cpool.tile([P, L, D], f32)
        nc.sync.dma_start(out=wv_st, in_=wv_d.rearrange("l d e -> d l e"))
        w1_st = cpool.tile([P, L, DFF], f32)
        nc.sync.dma_start(out=w1_st, in_=w1_d.rearrange("l d f -> d l f"))
        w2_st = cpool.tile([P, L, 4, D], f32)
        nc.sync.dma_start(out=w2_st, in_=w2_d.rearrange("l (c p) e -> p l c e", p=P))
        b1c_sb = cpool.tile([P, L, 4], f32)
        nc.sync.dma_start(out=b1c_sb, in_=b1_d.rearrange("l (c p) -> p l c", p=P))
        b2_col = cpool.tile([P, L], f32)
        nc.sync.dma_start(out=b2_col, in_=b2_d.rearrange("l d -> d l"))
        wout_st = cpool.tile([P, NCLS], f32)
        nc.sync.dma_start(out=wout_st, in_=wout_d[:, :])

        # Q/K bias slabs in slab partition order via partition-scatter DMA:
        # partition 32j+16u+dh holds b[l, 64g+16j+dh] (u duplicates).
        def bias_slab(src_d, nm):
            t = cpool.tile([P, L, NQUAD], f32, name=f"bslab{nm}")
            for g in range(NQUAD):
                for u in range(2):
                    src = bass.AP(
                        tensor=src_d[0, 0].tensor, offset=64 * g,
                        ap=[[16, 4], [1, DH], [D, L]],
                    )
                    nc.gpsimd.dma_start(
                        out=t[:, :, g].rearrange("(j u e) l -> j u e l",
                                                 j=4, u=2)[:, u, :, :],
                        in_=src,
                    )
            return t

        bq_sb = bias_slab(bq_d, "q")
        bk_sb = bias_slab(bk_d, "k")

        # partition-replicated per-feature vectors
        _repn = [0]

        def rep_load(src_ap, shape):
            _repn[0] += 1
            t = cpool.tile([P] + shape, f32, name=f"rep{_repn[0]}")
            bc = bass.AP(tensor=src_ap.tensor, offset=src_ap.offset,
                         ap=[[0, P]] + [list(e) for e in src_ap.ap])
            nc.gpsimd.dma_start(out=t, in_=bc)
            return t

        bv_rep = rep_load(bv_d[:, :], [L, D])
        l1b_rep = rep_load(l1b_d[:, :], [L, D])
        l1g_rep = rep_load(l1g_d[:, :], [L, D])
        l2g_rep = rep_load(l2g_d[:, :], [L, D])
        l2b_rep = rep_load(l2b_d[:, :], [L, D])
        bout_rep = rep_load(bout_d[:], [NCLS])

        # ================= prologue: on-chip casts to fp16 =================
        # Q/K weight slabs: quad g, head 4g+j at cols 32j..32j+15 AND
        # duplicated at 32j+16..31 (K=32 scores read both copies).
        wq_sb = cpool.tile([P, L, NQUAD, P], h16)
        wk_sb = cpool.tile([P, L, NQUAD, P], h16)
        for (w_st, w_sb) in ((wq_st, wq_sb), (wk_st, wk_sb)):
            for l in range(L):
                for g in range(NQUAD):
                    src = (w_st[:, l, 64 * g : 64 * g + 64]
                           .rearrange("d (j e) -> d j e", j=4)
                           [:, :, None, :].to_broadcast([P, 4, 2, DH]))
                    nc.vector.tensor_copy(
                        w_sb[:, l, g, :].rearrange("p (j u e) -> p j u e",
                                                   j=4, u=2),
                        src,
                    )
        wv_sb = cpool.tile([P, L, D], h16)
        nc.vector.tensor_copy(wv_sb, wv_st)
        w1_sb = cpool.tile([P, L, DFF], h16)
        nc.vector.tensor_copy(w1_sb, w1_st)
        w2_sb = cpool.tile([P, L, 4, D], h16)
        nc.vector.tensor_copy(w2_sb, w2_st)
        wout_sb = cpool.tile([P, NCLS], h16)
        nc.vector.tensor_copy(wout_sb, wout_st)

        # x in fp16, then x^T via one batched DMA transpose
        x16 = cpool.tile([P, TT, D], h16)
        nc.vector.tensor_copy(x16, x_sb)

        SC = 1.0 / np.sqrt(np.float32(DH))

        # ---------- helpers ----------
        def dma_T(out3, in3):
            """Batched XBAR transpose on the Sync HWDGE (fp16):
            out[do, di, m] = in[m, di, do]; in [M,Di,Do], out [Do,Di,M]."""
            nc.sync.dma_start(out=out3, in_=in3, transpose=True)

        def rsqrt_dve(rstd, var_ap, eps, w, tagp):
            """rstd = 1/sqrt(var+eps) on DVE (magic seed + 3 Newton steps)."""
            ve = small.tile([P, w], f32, tag="ve", name=f"ve{tagp}")
            nc.vector.tensor_scalar(out=ve, in0=var_ap, scalar1=float(eps),
                                    scalar2=None, op0=OP.add)
            yi = rstd.bitcast(i32)
            nc.vector.tensor_scalar(out=yi, in0=ve.bitcast(i32), scalar1=1,
                                    scalar2=None, op0=OP.logical_shift_right)
            nc.vector.tensor_scalar(out=yi, in0=yi, scalar1=0x5F3759DF,
                                    scalar2=-1, op0=OP.subtract, op1=OP.mult)
            nt = small.tile([P, w], f32, tag="nt", name=f"nt{tagp}")
            for _ in range(3):
                nc.vector.tensor_tensor(nt, rstd, rstd, OP.mult)
                nc.vector.tensor_tensor(nt, nt, ve, OP.mult)
                nc.vector.tensor_scalar(out=nt, in0=nt, scalar1=-0.5,
                                        scalar2=1.5, op0=OP.mult, op1=OP.add)
                nc.vector.tensor_tensor(rstd, rstd, nt, OP.mult)

        def alloc_layer_slabs(l):
            sl = {}
            sl["qt"] = apool.tile([P, NQUAD, TOK], h16, tag="qt", name=f"qt{l}")
            sl["kt"] = apool.tile([P, NQUAD, TOK], h16, tag="kt", name=f"kt{l}")
            sl["v"] = apool.tile([P, TT, H, 32], h16, tag="v", name=f"v{l}")
            nc.vector.memset(sl["v"][:, :, :, DH], 1.0)
            nc.vector.memset(sl["v"][:, :, :, DH + 1 : 32], 0.0)
            return sl

        def emit_qk_proj(l, sl, xt, proj, g, gq0):
            """One 512-token QK projection matmul + relu drain."""
            w_sb, b_sb, dston = (
                (wq_sb, bq_sb, "qt") if proj == 0 else (wk_sb, bk_sb, "kt"))
            pp = ps_mp.tile([P, 512], f32, tag="mp", name=f"pj{l}{proj}{g}{gq0}")
            nc.tensor.matmul(pp, w_sb[:, l, g, :], xt[:, gq0 : gq0 + 512],
                             start=True, stop=True)
            nc.vector.tensor_scalar(
                out=sl[dston][:, g, gq0 : gq0 + 512], in0=pp,
                scalar1=b_sb[:, l, g : g + 1], scalar2=0.0,
                op0=OP.add, op1=OP.max,
            )

        def emit_v_proj(l, sl, xt, t):
            """V projection for one 128-token tile + bias/relu drain."""
            pv = ps_mp.tile([P, D], f32, tag="mp", name=f"pv{l}{t}")
            nc.tensor.matmul(pv, xt[:, t * P : (t + 1) * P], wv_sb[:, l, :],
                             start=True, stop=True)
            nc.vector.tensor_tensor(
                sl["v"][:, t, :, 0:DH],
                pv.rearrange("p (h e) -> p h e", h=H),
                bv_rep[:, l, :].rearrange("p (h e) -> p h e", h=H),
                OP.add,
            )
            nc.vector.tensor_scalar(
                out=sl["v"][:, t, :, 0:DH], in0=sl["v"][:, t, :, 0:DH],
                scalar1=0.0, scalar2=None, op0=OP.max,
            )

        # ================= group pipeline stage pieces =================
        # Token group G=(b,qc): token tiles t0..t0+3, t0 = b*TPB + qc*NSUB.

        def ln_stats(src_tiles, mv, tagp):
            for i in range(NSUB):
                st6 = small.tile([P, 6], f32, tag="st6", name=f"st{tagp}{i}")
                nc.vector.bn_stats(out=st6, in_=src_tiles[:, i, :])
                nc.vector.bn_aggr(out=mv[:, i, :], in_=st6)

        def ln_norm(src, mv, rstd, dst):
            for i in range(NSUB):
                nc.vector.tensor_scalar(
                    out=dst[:, i, :], in0=src[:, i, :],
                    scalar1=mv[:, i, 0:1], scalar2=rstd[:, i : i + 1],
                    op0=OP.subtract, op1=OP.mult,
                )

        def s1_res(l, G, gst):
            res = gpool.tile([P, NSUB, D], f32, tag="res", name=f"res{l}{G}")
            mv = small.tile([P, NSUB, 2], f32, tag="mv", name=f"mv1{l}{G}")
            og, xprev_src = gst["og"], gst["xsrc"]
            b, qc = divmod(G, 2)
            t0 = b * TPB + qc * NSUB
            for i in range(NSUB):
                nc.vector.tensor_tensor(
                    res[:, i, :], og[:, i, :], xprev_src(t0 + i), OP.add)
            ln_stats(res, mv, f"1{l}{G}")
            gst.update(res=res, mv1=mv)

        def s1_ln(l, G, gst):
            rstd = small.tile([P, NSUB], f32, tag="rstd", name=f"rs1{l}{G}")
            rsqrt_dve(rstd, gst["mv1"][:, :, 1], 1e-8, NSUB, f"a{l}{G}")
            xn = gpool.tile([P, NSUB, D], f32, tag="xn", name=f"xn{l}{G}")
            ln_norm(gst["res"], gst["mv1"], rstd, xn)
            gst["xn"] = xn

        def s1_t1(l, G, gst, half):
            """t1 = xn*g1+b1 (fp16, gpsimd) for 2 tiles + their x1^T DMA."""
            if half == 0:
                gst["t1"] = gpool.tile([P, NSUB, D], h16, tag="t1",
                                       name=f"t1{l}{G}")
                gst["x1t"] = gpool.tile([P, NSUB, P], h16, tag="x1t",
                                        name=f"x1t{l}{G}")
            t1, xn = gst["t1"], gst["xn"]
            for i in (2 * half, 2 * half + 1):
                nc.gpsimd.tensor_tensor(
                    t1[:, i, :], xn[:, i, :], l1g_rep[:, l, :], OP.mult)
                nc.gpsimd.tensor_tensor(
                    t1[:, i, :], t1[:, i, :], l1b_rep[:, l, :], OP.add)
            dma_T(gst["x1t"][:, 2 * half : 2 * half + 2, :],
                  t1[:, 2 * half : 2 * half + 2, :])

        def s2_ffn1(l, G, gst, half):
            """FFN1 blocks 2h..2h+1: matmul + relu drain into ht."""
            if half == 0:
                gst["ht"] = gpool.tile([P, 4, 512], h16, tag="ht",
                                       name=f"ht{l}{G}")
            x1t = gst["x1t"].rearrange("d s m -> d (s m)")
            for c in (2 * half, 2 * half + 1):
                pp = ps_mp.tile([P, 512], f32, tag="mp", name=f"ph{l}{G}{c}")
                nc.tensor.matmul(pp, w1_sb[:, l, c * P : (c + 1) * P], x1t,
                                 start=True, stop=True)
                nc.vector.tensor_scalar(
                    out=gst["ht"][:, c, :], in0=pp,
                    scalar1=b1c_sb[:, l, c : c + 1], scalar2=0.0,
                    op0=OP.add, op1=OP.max,
                )

        def s2_ffn2(l, G, gst):
            """FFN2: 4 accumulating matmuls + b2 drain (fp16) + f^T DMA."""
            pf = ps_mp.tile([P, 512], f32, tag="mp", name=f"pf{l}{G}")
            for c in range(4):
                nc.tensor.matmul(pf, w2_sb[:, l, c, :], gst["ht"][:, c, :],
                                 start=(c == 0), stop=(c == 3))
            ft16 = gpool.tile([P, NSUB, P], h16, tag="ft16", name=f"ft{l}{G}")
            nc.vector.tensor_scalar(
                out=ft16.rearrange("d s m -> d (s m)"), in0=pf,
                scalar1=b2_col[:, l : l + 1], scalar2=None, op0=OP.add)
            ftt = gpool.tile([P, NSUB, P], h16, tag="ftt", name=f"ftt{l}{G}")
            dma_T(ftt, ft16)
            gst["ftt"] = ftt

        def s3_res2(l, G, gst):
            res2 = gpool.tile([P, NSUB, D], f32, tag="res2", name=f"re2{l}{G}")
            mv = small.tile([P, NSUB, 2], f32, tag="mv", name=f"mv2{l}{G}")
            for i in range(NSUB):
                nc.vector.tensor_tensor(
                    res2[:, i, :], gst["ftt"][:, i, :], gst["t1"][:, i, :],
                    OP.add)
            ln_stats(res2, mv, f"2{l}{G}")
            gst.update(res2=res2, mv2=mv)

        def s3_ln(l, G, gst):
            rstd = small.tile([P, NSUB], f32, tag="rstd", name=f"rs2{l}{G}")
            rsqrt_dve(rstd, gst["mv2"][:, :, 1], 1e-6, NSUB, f"b{l}{G}")
            xn2 = gpool.tile([P, NSUB, D], f32, tag="xn2", name=f"xn2{l}{G}")
            ln_norm(gst["res2"], gst["mv2"], rstd, xn2)
            gst["xn2"] = xn2

        def s3_xp(l, G, gst, xprev_next, xt_next, half):
            """xprev(l+1) = xn2*g2+b2 (fp16, gpsimd) + x^T(l+1) DMA."""
            xn2 = gst["xn2"]
            gq0 = G * 512
            for i in (2 * half, 2 * half + 1):
                nc.gpsimd.tensor_tensor(
                    xprev_next[:, i, :], xn2[:, i, :], l2g_rep[:, l, :],
                    OP.mult)
                nc.gpsimd.tensor_tensor(
                    xprev_next[:, i, :], xprev_next[:, i, :], l2b_rep[:, l, :],
                    OP.add)
            dma_T(xt_next[:, gq0 + 2 * half * P : gq0 + (2 * half + 2) * P]
                  .rearrange("d (t m) -> d t m", m=P),
                  xprev_next[:, 2 * half : 2 * half + 2, :])

        def emit_head(l, G, gst, xt):
            """Final 6-class projection for group G (4 tiny matmuls)."""
            b, qc = divmod(G, 2)
            t0 = b * TPB + qc * NSUB
            o6 = gpool.tile([P, NSUB, NCLS], f32, tag="o6", name=f"o6{G}")
            for i in range(NSUB):
                p6 = ps_mp.tile([P, NCLS], f32, tag="mp", name=f"p6{G}{i}")
                nc.tensor.matmul(p6, xt[:, (t0 + i) * P : (t0 + i + 1) * P],
                                 wout_sb, start=True, stop=True)
                nc.vector.tensor_tensor(o6[:, i, :], p6, bout_rep, OP.add)
            nc.sync.dma_start(
                out=out_d.rearrange("b (t p) c -> p (b t) c", p=P)
                [:, t0 : t0 + NSUB, :],
                in_=o6,
            )

        # ================= layer-0 x^T + projections (serial prologue) ====
        xt0 = apool.tile([P, TOK], h16, tag="xt", name="xt0")
        dma_T(xt0.rearrange("d (t m) -> d t m", m=P), x16)
        slabs0 = alloc_layer_slabs(0)
        for proj in range(2):
            for g in range(NQUAD):
                for gq0 in range(0, TOK, 512):
                    emit_qk_proj(0, slabs0, xt0, proj, g, gq0)
        for t in range(TT):
            emit_v_proj(0, slabs0, xt0, t)

        # ================= main pipelined layer loop =================
        fillers = deque()

        def pump(n):
            for _ in range(min(n, len(fillers))):
                fillers.popleft()()

        slabs_cur = slabs0
        xt_cur = xt0
        xprev_tiles = {}
        gstate = {}

        for l in range(L):
            last = l == L - 1
            if not last:
                slabs_next = alloc_layer_slabs(l + 1)
                xt_next = apool.tile([P, TOK], h16, tag="xt", name=f"xt{l+1}")
            else:
                slabs_next = None
                # final layer: xt_next holds (LN2*g2+b2)^T, the head input
                xt_next = apool.tile([P, TOK], h16, tag="xt", name="xtF")

            og_tiles = {}
            for ci, (b, qc, g) in enumerate(
                    [(b, qc, g) for b in range(B_LOC) for qc in range(2)
                     for g in range(NQUAD)]):
                G = b * 2 + qc
                qs0 = b * S + qc * QCW
                if g == 0:
                    og_tiles[G] = gpool.tile([P, NSUB, D], f32, tag="og",
                                             name=f"og{l}{G}")
                o_ps = ps_o.tile([P, QCW], f32, tag="o", name=f"o{l}{ci}")
                prev_e = None
                for kt in range(KT):
                    ks0 = b * S + kt * P
                    # score QUAD (K=32: head duplicated -> 2x score)
                    scps = [ps_sc.tile([P, 2, QCW], f32, tag="sc",
                                       name=f"sc{l}{ci}{kt}{pr}")
                            for pr in range(2)]
                    for j in range(4):
                        nc.tensor.matmul(
                            scps[j // 2][:, j % 2, :],
                            slabs_cur["kt"][32 * j : 32 * j + 32, g,
                                            ks0 : ks0 + P],
                            slabs_cur["qt"][32 * j : 32 * j + 32, g,
                                            qs0 : qs0 + QCW],
                            start=True, stop=True,
                            tile_position=(32 * j, 0),
                        )
                    cur_e = []
                    for pr in range(2):
                        e_sb = epool.tile([P, 2, QCW], h16, tag="e",
                                          name=f"e{l}{ci}{kt}{pr}")
                        nc.scalar.activation(
                            out=e_sb.rearrange("p a q -> p (a q)"),
                            in_=scps[pr].rearrange("p a q -> p (a q)"),
                            func=AF.Exp, scale=float(SC / 2),
                        )
                        cur_e.append(e_sb)
                    if prev_e is not None:
                        pkt, pe0, pe1 = prev_e
                        for j in range(4):
                            nc.tensor.matmul(
                                o_ps[32 * j : 32 * j + 32, :],
                                slabs_cur["v"][:, b * TPB + pkt, 4 * g + j, :],
                                (pe0 if j < 2 else pe1)[:, j % 2, :],
                                start=(pkt == 0), stop=False,
                                tile_position=(0, 32 * j),
                                skip_group_check=True,
                            )
                    prev_e = (kt, cur_e[0], cur_e[1])
                    pump(1)
                pkt, pe0, pe1 = prev_e
                for j in range(4):
                    nc.tensor.matmul(
                        o_ps[32 * j : 32 * j + 32, :],
                        slabs_cur["v"][:, b * TPB + pkt, 4 * g + j, :],
                        (pe0 if j < 2 else pe1)[:, j % 2, :],
                        start=False, stop=True,
                        tile_position=(0, 32 * j),
                        skip_group_check=True,
                    )
                # chunk epilogue: drain -> DMA transpose -> normalize
                ot16 = small.tile([P, QCW], h16, tag="ot16", name=f"ot{l}{ci}")
                nc.vector.tensor_copy(ot16, o_ps)
                ott = gpool.tile([P, NSUB, P], h16, tag="ott",
                                 name=f"ott{l}{ci}")
                dma_T(ott, ot16.rearrange("p (s m) -> p s m", m=P))
                rcp = small.tile([P, NSUB, 4], f32, tag="rcp",
                                 name=f"rcp{l}{ci}")
                nc.vector.reciprocal(rcp, ott[:, :, DH :: 32])
                og = og_tiles[G]
                nc.vector.tensor_tensor(
                    og[:, :, 64 * g : 64 * g + 64]
                        .rearrange("p s (j e) -> p s j e", j=4),
                    ott.rearrange("p s (j u) -> p s j u", j=4)[:, :, :, 0:DH],
                    rcp[:, :, :, None].to_broadcast([P, NSUB, 4, DH]),
                    OP.mult,
                )

                # after the 2nd quad of a group, enqueue its 16-pop pipeline
                if g == 1:
                    gst = gstate.setdefault((l, G), {})
                    gst["og"] = og_tiles[G]
                    if l == 0:
                        gst["xsrc"] = lambda t: x_sb[:, t, :]
                    else:
                        xp = xprev_tiles[(l, G)]
                        b_, qc_ = divmod(G, 2)
                        t0_ = b_ * TPB + qc_ * NSUB
                        gst["xsrc"] = lambda t, xp=xp, t0_=t0_: xp[:, t - t0_, :]
                    E = fillers.append
                    E(lambda l=l, G=G, gst=gst: s1_res(l, G, gst))
                    E(lambda l=l, G=G, gst=gst: s1_ln(l, G, gst))
                    E(lambda l=l, G=G, gst=gst: s1_t1(l, G, gst, 0))
                    E(lambda l=l, G=G, gst=gst: s1_t1(l, G, gst, 1))
                    E(lambda l=l, G=G, gst=gst: s2_ffn1(l, G, gst, 0))
                    E(lambda l=l, G=G, gst=gst: s2_ffn1(l, G, gst, 1))
                    E(lambda l=l, G=G, gst=gst: s2_ffn2(l, G, gst))
                    E(lambda l=l, G=G, gst=gst: s3_res2(l, G, gst))
                    E(lambda l=l, G=G, gst=gst: s3_ln(l, G, gst))
                    xp_next = xpool.tile([P, NSUB, D], h16, tag="xp",
                                         name=f"xp{l+1}{G}")
                    if not last:
                        xprev_tiles[(l + 1, G)] = xp_next
                    E(lambda l=l, G=G, gst=gst, xp=xp_next, xt=xt_next:
                      s3_xp(l, G, gst, xp, xt, 0))
                    E(lambda l=l, G=G, gst=gst, xp=xp_next, xt=xt_next:
                      s3_xp(l, G, gst, xp, xt, 1))
                    if not last:
                        sl_n, xt_n = slabs_next, xt_next
                        for proj in range(2):
                            for gg in range(NQUAD):
                                E(lambda l=l, G=G, sl=sl_n, xt=xt_n,
                                    proj=proj, gg=gg:
                                  emit_qk_proj(l + 1, sl, xt, proj, gg,
                                               G * 512))
                        b_, qc_ = divmod(G, 2)
                        t0 = b_ * TPB + qc_ * NSUB
                        for i in range(NSUB):
                            E(lambda l=l, sl=sl_n, xt=xt_n, t=t0 + i:
                              emit_v_proj(l + 1, sl, xt, t))
                    else:
                        E(lambda l=l, G=G, gst=gst, xt=xt_next:
                          emit_head(l, G, gst, xt))

            slabs_cur = slabs_next
            xt_cur = xt_next

        pump(len(fillers))
        ctx.close()

    nc.compile()
    return nc


def _get_nc():
    if "nc" not in _CACHE:
        _CACHE["nc"] = _build_nc()
    return _CACHE["nc"]


def kernel(**inputs) -> np.ndarray:
    from concourse.bass_utils import run_bass_kernel_spmd

    nc = _get_nc()
    ins = {k: np.ascontiguousarray(np.asarray(v)) for k, v in inputs.items()}
    in_maps = []
    for c in range(NCORES):
        m = dict(ins)
        m["x"] = np.ascontiguousarray(ins["x"][c * B_LOC : (c + 1) * B_LOC])
        in_maps.append(m)
    res = run_bass_kernel_spmd(nc, in_maps, list(range(NCORES)))
    out = np.concatenate([res.results[c]["out"] for c in range(NCORES)], axis=0)
    return out


# revision 24
# speedup vs baseline: 1.4458x; 1.0254x over previous
"""Trainium2 Bass kernel for nn_AttentionModel_87462714015827.

3-layer transformer encoder: B=16, S=1024, D=128, H=8 heads (DH=16),
FFN hidden 512, final 6-class projection.

Sharding: data-parallel over batch across 8 NeuronCores (2 batches/core),
all parameters replicated, no collectives.

v2 architecture:
  - ScalarE (ACT) does NOTHING but softmax exp; it is the kernel's
    metronome (~1.11us per 2-PSUM-bank exp, ~143us/layer).
  - All matmul streams are fp16 (f32 PSUM accumulate; fp16's 10-bit
    mantissa keeps the end-to-end error ~8x below bf16). Weights are
    DMA'd f32 and cast on-chip in the prologue.
  - NO PE transposes: every layout flip (x^T, x1^T, o, f) is a hardware
    XBAR DMA transpose issued on the otherwise-idle Sync engine.
  - PSUM: score pool 3 bufs x 2 banks (the 4 score matmuls of a kt
    issue back-to-back and run 4-way row-tiled concurrent), o 1 bank,
    mp 1 bank for all projection/FFN matmuls.
  - The LN/FFN/projection pipeline for each 512-token group is emitted
    as 16 small closures ("fillers"), popped one per kt iteration of
    the attention loop (64 slots = 4 groups x 16 per layer), so that
    work flows inside the PE/DVE slack under the continuous exp stream.
  - Scores use K=32 (each head duplicated in the slab) -> 2x score,
    folded into the exp scale.
"""

import os
import sys
from collections import deque

import numpy as np

for _p in ("/opt/trn_rl_repo", "/root/.axon_site/_ro/trn_rl_repo"):
    if os.path.isdir(_p) and _p not in sys.path:
        sys.path.insert(0, _p)

B, S, D, H, L = 16, 1024, 128, 8, 3
DFF = 4 * D          # 512
DH = D // H          # 16
NCLS = 6
NCORES = 8
B_LOC = B // NCORES  # 2
TOK = B_LOC * S      # 2048
TT = TOK // 128      # 16 token tiles per core
TPB = S // 128       # 8 token tiles per batch
P = 128
NQUAD = 2            # head quads (4 heads each)
KT = TPB             # 8 k tiles of 128 per batch
QCW = 512            # q-chunk width
NSUB = QCW // P      # 4 token tiles per group
NG = 4               # token groups (b, qc) per layer

_CACHE = {}


def _build_nc():
    import concourse.bass as bass
    import concourse.mybir as mybir
    import concourse.tile as tile
    from concourse import bacc

    dt = mybir.dt
    f32 = dt.float32
    h16 = dt.float16
    i32 = dt.int32
    AF = mybir.ActivationFunctionType
    OP = mybir.AluOpType

    nc = bacc.Bacc("TRN2", target_bir_lowering=False)

    # ---- DRAM I/O ----
    x_d = nc.dram_tensor("x", [B_LOC, S, D], f32, kind="ExternalInput")
    wq_d = nc.dram_tensor("Wq", [L, D, D], f32, kind="ExternalInput")
    bq_d = nc.dram_tensor("bq", [L, D], f32, kind="ExternalInput")
    wk_d = nc.dram_tensor("Wk", [L, D, D], f32, kind="ExternalInput")
    bk_d = nc.dram_tensor("bk", [L, D], f32, kind="ExternalInput")
    wv_d = nc.dram_tensor("Wv", [L, D, D], f32, kind="ExternalInput")
    bv_d = nc.dram_tensor("bv", [L, D], f32, kind="ExternalInput")
    l1g_d = nc.dram_tensor("ln1_g", [L, D], f32, kind="ExternalInput")
    l1b_d = nc.dram_tensor("ln1_b", [L, D], f32, kind="ExternalInput")
    w1_d = nc.dram_tensor("W1", [L, D, DFF], f32, kind="ExternalInput")
    b1_d = nc.dram_tensor("b1", [L, DFF], f32, kind="ExternalInput")
    w2_d = nc.dram_tensor("W2", [L, DFF, D], f32, kind="ExternalInput")
    b2_d = nc.dram_tensor("b2", [L, D], f32, kind="ExternalInput")
    l2g_d = nc.dram_tensor("ln2_g", [L, D], f32, kind="ExternalInput")
    l2b_d = nc.dram_tensor("ln2_b", [L, D], f32, kind="ExternalInput")
    wout_d = nc.dram_tensor("Wout", [D, NCLS], f32, kind="ExternalInput")
    bout_d = nc.dram_tensor("bout", [NCLS], f32, kind="ExternalInput")
    out_d = nc.dram_tensor("out", [B_LOC, S, NCLS], f32, kind="ExternalOutput")

    with tile.TileContext(nc) as tc:
        from contextlib import ExitStack

        ctx = ExitStack()
        cpool = ctx.enter_context(tc.tile_pool(name="const", bufs=1))
        apool = ctx.enter_context(tc.tile_pool(name="acts", bufs=2))
        gpool = ctx.enter_context(tc.tile_pool(name="grp", bufs=2))
        xpool = ctx.enter_context(tc.tile_pool(name="xprev", bufs=6))
        epool = ctx.enter_context(tc.tile_pool(name="epool", bufs=6))
        small = ctx.enter_context(tc.tile_pool(name="small", bufs=2))
        ps_sc = ctx.enter_context(tc.tile_pool(name="ps_sc", bufs=3, space="PSUM"))
        ps_o = ctx.enter_context(tc.tile_pool(name="ps_o", bufs=1, space="PSUM"))
        ps_mp = ctx.enter_context(tc.tile_pool(name="ps_mp", bufs=1, space="PSUM"))

        # HAM warmup first: no DMA deps, opens the PE clock gate at t=0.
        wu16 = cpool.tile([P, 512], h16)
        nc.vector.memset(wu16, 1.0)
        wup = ps_mp.tile([P, 512], f32, tag="mp", name="wup")
        for w in range(8):
            nc.tensor.matmul(wup, wu16[:, 0:P], wu16, start=True, stop=True)

        # ================= prologue: loads =================
        # x goes on the ACT HWDGE queue so it doesn't wait behind the
        # weight staging loads on the Sync queue (ACT is idle here).
        x_sb = cpool.tile([P, TT, D], f32)
        nc.scalar.dma_start(out=x_sb, in_=x_d.rearrange("b (t p) d -> p (b t) d", p=P))

        wq_st = cpool.tile([P, L, D], f32)
        nc.sync.dma_start(out=wq_st, in_=wq_d.rearrange("l d e -> d l e"))
        wk_st = cpool.tile([P, L, D], f32)
        nc.sync.dma_start(out=wk_st, in_=wk_d.rearrange("l d e -> d l e"))
        wv_st = cpool.tile([P, L, D], f32)
        nc.sync.dma_start(out=wv_st, in_=wv_d.rearrange("l d e -> d l e"))
        w1_st = cpool.tile([P, L, DFF], f32)
        nc.sync.dma_start(out=w1_st, in_=w1_d.rearrange("l d f -> d l f"))
        w2_st = cpool.tile([P, L, 4, D], f32)
        nc.sync.dma_start(out=w2_st, in_=w2_d.rearrange("l (c p) e -> p l c e", p=P))
        b1c_sb = cpool.tile([P, L, 4], f32)
        nc.sync.dma_start(out=b1c_sb, in_=b1_d.rearrange("l (c p) -> p l c", p=P))
        b2_col = cpool.tile([P, L], f32)
        nc.sync.dma_start(out=b2_col, in_=b2_d.rearrange("l d -> d l"))
        wout_st = cpool.tile([P, NCLS], f32)
        nc.sync.dma_start(out=wout_st, in_=wout_d[:, :])

        # Q/K bias slabs in slab partition order via partition-scatter DMA:
        # partition 32j+16u+dh holds b[l, 64g+16j+dh] (u duplicates).
        def bias_slab(src_d, nm):
            t = cpool.tile([P, L, NQUAD], f32, name=f"bslab{nm}")
            for g in range(NQUAD):
                for u in range(2):
                    src = bass.AP(
                        tensor=src_d[0, 0].tensor, offset=64 * g,
                        ap=[[16, 4], [1, DH], [D, L]],
                    )
                    nc.gpsimd.dma_start(
                        out=t[:, :, g].rearrange("(j u e) l -> j u e l",
                                                 j=4, u=2)[:, u, :, :],
                        in_=src,
                    )
            return t

        bq_sb = bias_slab(bq_d, "q")
        bk_sb = bias_slab(bk_d, "k")

        # partition-replicated per-feature vectors
        _repn = [0]

        def rep_load(src_ap, shape):
            _repn[0] += 1
            t = cpool.tile([P] + shape, f32, name=f"rep{_repn[0]}")
            bc = bass.AP(tensor=src_ap.tensor, offset=src_ap.offset,
                         ap=[[0, P]] + [list(e) for e in src_ap.ap])
            nc.gpsimd.dma_start(out=t, in_=bc)
            return t

        bv_rep = rep_load(bv_d[:, :], [L, D])
        l1b_rep = rep_load(l1b_d[:, :], [L, D])
        l1g_rep = rep_load(l1g_d[:, :], [L, D])
        l2g_rep = rep_load(l2g_d[:, :], [L, D])
        l2b_rep = rep_load(l2b_d[:, :], [L, D])
        bout_rep = rep_load(bout_d[:], [NCLS])

        # ================= prologue: on-chip casts to fp16 =================
        # Q/K weight slabs: quad g, head 4g+j at cols 32j..32j+15 AND
        # duplicated at 32j+16..31 (K=32 scores read both copies).
        wq_sb = cpool.tile([P, L, NQUAD, P], h16)
        wk_sb = cpool.tile([P, L, NQUAD, P], h16)
        for (w_st, w_sb) in ((wq_st, wq_sb), (wk_st, wk_sb)):
            for l in range(L):
                for g in range(NQUAD):
                    src = (w_st[:, l, 64 * g : 64 * g + 64]
                           .rearrange("d (j e) -> d j e", j=4)
                           [:, :, None, :].to_broadcast([P, 4, 2, DH]))
                    nc.vector.tensor_copy(
                        w_sb[:, l, g, :].rearrange("p (j u e) -> p j u e",
                                                   j=4, u=2),
                        src,
                    )
        wv_sb = cpool.tile([P, L, D], h16)
        nc.vector.tensor_copy(wv_sb, wv_st)
        w1_sb = cpool.tile([P, L, DFF], h16)
        nc.vector.tensor_copy(w1_sb, w1_st)
        w2_sb = cpool.tile([P, L, 4, D], h16)
        nc.vector.tensor_copy(w2_sb, w2_st)
        wout_sb = cpool.tile([P, NCLS], h16)
        nc.vector.tensor_copy(wout_sb, wout_st)

        # x in fp16, then x^T via one batched DMA transpose (ACT queue)
        x16 = cpool.tile([P, TT, D], h16)
        nc.vector.tensor_copy(x16, x_sb)

        SC = 1.0 / np.sqrt(np.float32(DH))

        # ---------- helpers ----------
        def dma_T(out3, in3):
            """Batched XBAR transpose on the Sync HWDGE (fp16):
            out[do, di, m] = in[m, di, do]; in [M,Di,Do], out [Do,Di,M]."""
            nc.sync.dma_start(out=out3, in_=in3, transpose=True)

        def rsqrt_dve(rstd, var_ap, eps, w, tagp):
            """rstd = 1/sqrt(var+eps) on DVE (magic seed + 3 Newton steps)."""
            ve = small.tile([P, w], f32, tag="ve", name=f"ve{tagp}")
            nc.vector.tensor_scalar(out=ve, in0=var_ap, scalar1=float(eps),
                                    scalar2=None, op0=OP.add)
            yi = rstd.bitcast(i32)
            nc.vector.tensor_scalar(out=yi, in0=ve.bitcast(i32), scalar1=1,
                                    scalar2=None, op0=OP.logical_shift_right)
            nc.vector.tensor_scalar(out=yi, in0=yi, scalar1=0x5F3759DF,
                                    scalar2=-1, op0=OP.subtract, op1=OP.mult)
            nt = small.tile([P, w], f32, tag="nt", name=f"nt{tagp}")
            for _ in range(3):
                nc.vector.tensor_tensor(nt, rstd, rstd, OP.mult)
                nc.vector.tensor_tensor(nt, nt, ve, OP.mult)
                nc.vector.tensor_scalar(out=nt, in0=nt, scalar1=-0.5,
                                        scalar2=1.5, op0=OP.mult, op1=OP.add)
                nc.vector.tensor_tensor(rstd, rstd, nt, OP.mult)

        def alloc_layer_slabs(l):
            sl = {}
            sl["qt"] = apool.tile([P, NQUAD, TOK], h16, tag="qt", name=f"qt{l}")
            sl["kt"] = apool.tile([P, NQUAD, TOK], h16, tag="kt", name=f"kt{l}")
            sl["v"] = apool.tile([P, TT, H, 32], h16, tag="v", name=f"v{l}")
            nc.vector.memset(sl["v"][:, :, :, DH], 1.0)
            nc.vector.memset(sl["v"][:, :, :, DH + 1 : 32], 0.0)
            return sl

        def emit_qk_proj(l, sl, xt, proj, g, gq0):
            """One 512-token QK projection matmul + relu drain."""
            w_sb, b_sb, dston = (
                (wq_sb, bq_sb, "qt") if proj == 0 else (wk_sb, bk_sb, "kt"))
            pp = ps_mp.tile([P, 512], f32, tag="mp", name=f"pj{l}{proj}{g}{gq0}")
            nc.tensor.matmul(pp, w_sb[:, l, g, :], xt[:, gq0 : gq0 + 512],
                             start=True, stop=True)
            nc.vector.tensor_scalar(
                out=sl[dston][:, g, gq0 : gq0 + 512], in0=pp,
                scalar1=b_sb[:, l, g : g + 1], scalar2=0.0,
                op0=OP.add, op1=OP.max,
            )

        def emit_v_proj(l, sl, xt, t):
            """V projection for one 128-token tile + bias/relu drain."""
            pv = ps_mp.tile([P, D], f32, tag="mp", name=f"pv{l}{t}")
            nc.tensor.matmul(pv, xt[:, t * P : (t + 1) * P], wv_sb[:, l, :],
                             start=True, stop=True)
            nc.vector.tensor_tensor(
                sl["v"][:, t, :, 0:DH],
                pv.rearrange("p (h e) -> p h e", h=H),
                bv_rep[:, l, :].rearrange("p (h e) -> p h e", h=H),
                OP.add,
            )
            nc.vector.tensor_scalar(
                out=sl["v"][:, t, :, 0:DH], in0=sl["v"][:, t, :, 0:DH],
                scalar1=0.0, scalar2=None, op0=OP.max,
            )

        # ================= group pipeline stage pieces =================
        # Token group G=(b,qc): token tiles t0..t0+3, t0 = b*TPB + qc*NSUB.

        def ln_stats(src_tiles, mv, tagp):
            for i in range(NSUB):
                st6 = small.tile([P, 6], f32, tag="st6", name=f"st{tagp}{i}")
                nc.vector.bn_stats(out=st6, in_=src_tiles[:, i, :])
                nc.vector.bn_aggr(out=mv[:, i, :], in_=st6)

        def ln_norm(src, mv, rstd, dst):
            for i in range(NSUB):
                nc.vector.tensor_scalar(
                    out=dst[:, i, :], in0=src[:, i, :],
                    scalar1=mv[:, i, 0:1], scalar2=rstd[:, i : i + 1],
                    op0=OP.subtract, op1=OP.mult,
                )

        def s1_res(l, G, gst):
            res = gpool.tile([P, NSUB, D], f32, tag="res", name=f"res{l}{G}")
            mv = small.tile([P, NSUB, 2], f32, tag="mv", name=f"mv1{l}{G}")
            og, xprev_src = gst["og"], gst["xsrc"]
            b, qc = divmod(G, 2)
            t0 = b * TPB + qc * NSUB
            for i in range(NSUB):
                nc.vector.tensor_tensor(
                    res[:, i, :], og[:, i, :], xprev_src(t0 + i), OP.add)
            ln_stats(res, mv, f"1{l}{G}")
            gst.update(res=res, mv1=mv)

        def s1_ln(l, G, gst):
            rstd = small.tile([P, NSUB], f32, tag="rstd", name=f"rs1{l}{G}")
            rsqrt_dve(rstd, gst["mv1"][:, :, 1], 1e-8, NSUB, f"a{l}{G}")
            xn = gpool.tile([P, NSUB, D], f32, tag="xn", name=f"xn{l}{G}")
            ln_norm(gst["res"], gst["mv1"], rstd, xn)
            gst["xn"] = xn

        def s1_t1(l, G, gst, half):
            """t1 = xn*g1+b1 (fp16, gpsimd) for 2 tiles + their x1^T DMA."""
            if half == 0:
                gst["t1"] = gpool.tile([P, NSUB, D], h16, tag="t1",
                                       name=f"t1{l}{G}")
                gst["x1t"] = gpool.tile([P, NSUB, P], h16, tag="x1t",
                                        name=f"x1t{l}{G}")
            t1, xn = gst["t1"], gst["xn"]
            for i in (2 * half, 2 * half + 1):
                nc.gpsimd.tensor_tensor(
                    t1[:, i, :], xn[:, i, :], l1g_rep[:, l, :], OP.mult)
                nc.gpsimd.tensor_tensor(
                    t1[:, i, :], t1[:, i, :], l1b_rep[:, l, :], OP.add)
            dma_T(gst["x1t"][:, 2 * half : 2 * half + 2, :],
                  t1[:, 2 * half : 2 * half + 2, :])

        def s2_ffn1(l, G, gst, half):
            """FFN1 blocks 2h..2h+1: matmul + relu drain into ht."""
            if half == 0:
                gst["ht"] = gpool.tile([P, 4, 512], h16, tag="ht",
                                       name=f"ht{l}{G}")
            x1t = gst["x1t"].rearrange("d s m -> d (s m)")
            for c in (2 * half, 2 * half + 1):
                pp = ps_mp.tile([P, 512], f32, tag="mp", name=f"ph{l}{G}{c}")
                nc.tensor.matmul(pp, w1_sb[:, l, c * P : (c + 1) * P], x1t,
                                 start=True, stop=True)
                nc.vector.tensor_scalar(
                    out=gst["ht"][:, c, :], in0=pp,
                    scalar1=b1c_sb[:, l, c : c + 1], scalar2=0.0,
                    op0=OP.add, op1=OP.max,
                )

        def s2_ffn2(l, G, gst):
            """FFN2: 4 accumulating matmuls + b2 drain (fp16) + f^T DMA."""
            pf = ps_mp.tile([P, 512], f32, tag="mp", name=f"pf{l}{G}")
            for c in range(4):
                nc.tensor.matmul(pf, w2_sb[:, l, c, :], gst["ht"][:, c, :],
                                 start=(c == 0), stop=(c == 3))
            ft16 = gpool.tile([P, NSUB, P], h16, tag="ft16", name=f"ft{l}{G}")
            nc.vector.tensor_scalar(
                out=ft16.rearrange("d s m -> d (s m)"), in0=pf,
                scalar1=b2_col[:, l : l + 1], scalar2=None, op0=OP.add)
            ftt = gpool.tile([P, NSUB, P], h16, tag="ftt", name=f"ftt{l}{G}")
            dma_T(ftt, ft16)
            gst["ftt"] = ftt

        def s3_res2(l, G, gst):
            res2 = gpool.tile([P, NSUB, D], f32, tag="res2", name=f"re2{l}{G}")
            mv = small.tile([P, NSUB, 2], f32, tag="mv", name=f"mv2{l}{G}")
            for i in range(NSUB):
                nc.vector.tensor_tensor(
                    res2[:, i, :], gst["ftt"][:, i, :], gst["t1"][:, i, :],
                    OP.add)
            ln_stats(res2, mv, f"2{l}{G}")
            gst.update(res2=res2, mv2=mv)

        def s3_ln(l, G, gst):
            rstd = small.tile([P, NSUB], f32, tag="rstd", name=f"rs2{l}{G}")
            rsqrt_dve(rstd, gst["mv2"][:, :, 1], 1e-6, NSUB, f"b{l}{G}")
            xn2 = gpool.tile([P, NSUB, D], f32, tag="xn2", name=f"xn2{l}{G}")
            ln_norm(gst["res2"], gst["mv2"], rstd, xn2)
            gst["xn2"] = xn2

        def s3_xp(l, G, gst, xprev_next, xt_next, half):
            """xprev(l+1) = xn2*g2+b2 (fp16, gpsimd) + x^T(l+1) DMA."""
            xn2 = gst["xn2"]
            gq0 = G * 512
            for i in (2 * half, 2 * half + 1):
                nc.gpsimd.tensor_tensor(
                    xprev_next[:, i, :], xn2[:, i, :], l2g_rep[:, l, :],
                    OP.mult)
                nc.gpsimd.tensor_tensor(
                    xprev_next[:, i, :], xprev_next[:, i, :], l2b_rep[:, l, :],
                    OP.add)
            dma_T(xt_next[:, gq0 + 2 * half * P : gq0 + (2 * half + 2) * P]
                  .rearrange("d (t m) -> d t m", m=P),
                  xprev_next[:, 2 * half : 2 * half + 2, :])

        def emit_head(l, G, gst, xt):
            """Final 6-class projection for group G (4 tiny matmuls)."""
            b, qc = divmod(G, 2)
            t0 = b * TPB + qc * NSUB
            o6 = gpool.tile([P, NSUB, NCLS], f32, tag="o6", name=f"o6{G}")
            for i in range(NSUB):
                p6 = ps_mp.tile([P, NCLS], f32, tag="mp", name=f"p6{G}{i}")
                nc.tensor.matmul(p6, xt[:, (t0 + i) * P : (t0 + i + 1) * P],
                                 wout_sb, start=True, stop=True)
                nc.vector.tensor_tensor(o6[:, i, :], p6, bout_rep, OP.add)
            nc.sync.dma_start(
                out=out_d.rearrange("b (t p) c -> p (b t) c", p=P)
                [:, t0 : t0 + NSUB, :],
                in_=o6,
            )

        # ================= layer-0 x^T + projections (serial prologue) ====
        # x^T transpose on the ACT HWDGE queue (no waiting behind weights);
        # projections via the (still free) score pool, double-buffered.
        xt0 = apool.tile([P, TOK], h16, tag="xt", name="xt0")
        nc.scalar.dma_start(out=xt0.rearrange("d (t m) -> d t m", m=P),
                            in_=x16, transpose=True)
        slabs0 = alloc_layer_slabs(0)
        for g in range(NQUAD):
            for proj in range(2):
                w_sb, b_sb, dston = (
                    (wq_sb, bq_sb, "qt") if proj == 0 else (wk_sb, bk_sb, "kt"))
                for cp in range(2):
                    sct = ps_sc.tile([P, 2, QCW], f32, tag="sc",
                                     name=f"p0{proj}{g}{cp}")
                    for h in range(2):
                        nc.tensor.matmul(
                            sct[:, h, :], w_sb[:, 0, g, :],
                            xt0[:, cp * 1024 + h * 512 : cp * 1024 + (h + 1) * 512],
                            start=True, stop=True)
                    nc.vector.tensor_scalar(
                        out=slabs0[dston][:, g, cp * 1024 : (cp + 1) * 1024],
                        in0=sct.rearrange("p a q -> p (a q)"),
                        scalar1=b_sb[:, 0, g : g + 1], scalar2=0.0,
                        op0=OP.add, op1=OP.max,
                    )
        for tp in range(TT // 2):
            sct = ps_sc.tile([P, 2, QCW], f32, tag="sc", name=f"pv0{tp}")
            for h in range(2):
                t = 2 * tp + h
                nc.tensor.matmul(sct[:, h, 0:D],
                                 xt0[:, t * P : (t + 1) * P], wv_sb[:, 0, :],
                                 start=True, stop=True)
            for h in range(2):
                t = 2 * tp + h
                nc.vector.tensor_tensor(
                    slabs0["v"][:, t, :, 0:DH],
                    sct[:, h, 0:D].rearrange("p (h2 e) -> p h2 e", h2=H),
                    bv_rep[:, 0, :].rearrange("p (h2 e) -> p h2 e", h2=H),
                    OP.add,
                )
                nc.vector.tensor_scalar(
                    out=slabs0["v"][:, t, :, 0:DH],
                    in0=slabs0["v"][:, t, :, 0:DH],
                    scalar1=0.0, scalar2=None, op0=OP.max,
                )

        # ================= main pipelined layer loop =================
        fillers = deque()

        def pump():
            n = 2 if len(fillers) > 18 else 1
            for _ in range(min(n, len(fillers))):
                fillers.popleft()()

        PAD = lambda: None
        slabs_cur = slabs0
        xprev_tiles = {}
        gstate = {}
        pending = [None]  # lagged attnv(+epilogue) of the previous kt step

        for l in range(L):
            last = l == L - 1
            if not last:
                slabs_next = alloc_layer_slabs(l + 1)
                xt_next = apool.tile([P, TOK], h16, tag="xt", name=f"xt{l+1}")
            else:
                slabs_next = None
                # final layer: xt_next holds (LN2*g2+b2)^T, the head input
                xt_next = apool.tile([P, TOK], h16, tag="xt", name="xtF")

            og_tiles = {}
            for ci, (b, qc, g) in enumerate(
                    [(b, qc, g) for b in range(B_LOC) for qc in range(2)
                     for g in range(NQUAD)]):
                G = b * 2 + qc
                qs0 = b * S + qc * QCW
                if g == 0:
                    og_tiles[G] = gpool.tile([P, NSUB, D], f32, tag="og",
                                             name=f"og{l}{G}")

                def make_epi(l=l, ci=ci, b=b, qc=qc, g=g, G=G,
                             o_ps=None, og=None, slabs_next=slabs_next,
                             xt_next=xt_next, last=last):
                    def epi():
                        ot16 = small.tile([P, QCW], h16, tag="ot16",
                                          name=f"ot{l}{ci}")
                        nc.vector.tensor_copy(ot16, o_ps)
                        ott = gpool.tile([P, NSUB, P], h16, tag="ott",
                                         name=f"ott{l}{ci}")
                        dma_T(ott, ot16.rearrange("p (s m) -> p s m", m=P))
                        rcp = small.tile([P, NSUB, 4], f32, tag="rcp",
                                         name=f"rcp{l}{ci}")
                        nc.vector.reciprocal(rcp, ott[:, :, DH :: 32])
                        nc.vector.tensor_tensor(
                            og[:, :, 64 * g : 64 * g + 64]
                                .rearrange("p s (j e) -> p s j e", j=4),
                            ott.rearrange("p s (j u) -> p s j u", j=4)
                            [:, :, :, 0:DH],
                            rcp[:, :, :, None].to_broadcast([P, NSUB, 4, DH]),
                            OP.mult,
                        )
                        if g != 1:
                            return
                        # enqueue this group's 16-pop F/P1 pipeline
                        gst = gstate.setdefault((l, G), {})
                        gst["og"] = og
                        if l == 0:
                            gst["xsrc"] = lambda t: x_sb[:, t, :]
                        else:
                            xp = xprev_tiles[(l, G)]
                            b_, qc_ = divmod(G, 2)
                            t0_ = b_ * TPB + qc_ * NSUB
                            gst["xsrc"] = (
                                lambda t, xp=xp, t0_=t0_: xp[:, t - t0_, :])
                        E = fillers.append
                        E(lambda: s1_res(l, G, gst))
                        E(lambda: s1_ln(l, G, gst))
                        E(lambda: s1_t1(l, G, gst, 0))
                        E(lambda: s1_t1(l, G, gst, 1))
                        E(PAD)
                        E(lambda: s2_ffn1(l, G, gst, 0))
                        E(lambda: s2_ffn1(l, G, gst, 1))
                        E(lambda: s2_ffn2(l, G, gst))
                        E(lambda: (s3_res2(l, G, gst), s3_ln(l, G, gst)))
                        xp_next = xpool.tile([P, NSUB, D], h16, tag="xp",
                                             name=f"xp{l+1}{G}")
                        if not last:
                            xprev_tiles[(l + 1, G)] = xp_next
                        E(lambda: s3_xp(l, G, gst, xp_next, xt_next, 0))
                        E(lambda: s3_xp(l, G, gst, xp_next, xt_next, 1))
                        if not last:
                            b_, qc_ = divmod(G, 2)
                            t0 = b_ * TPB + qc_ * NSUB
                            E(lambda: (emit_v_proj(l + 1, slabs_next, xt_next, t0),
                                       emit_v_proj(l + 1, slabs_next, xt_next, t0 + 1)))
                            E(lambda: (emit_v_proj(l + 1, slabs_next, xt_next, t0 + 2),
                                       emit_v_proj(l + 1, slabs_next, xt_next, t0 + 3)))
                            E(lambda: (emit_qk_proj(l + 1, slabs_next, xt_next, 0, 0, G * 512),
                                       emit_qk_proj(l + 1, slabs_next, xt_next, 1, 0, G * 512)))
                            E(lambda: (emit_qk_proj(l + 1, slabs_next, xt_next, 0, 1, G * 512),
                                       emit_qk_proj(l + 1, slabs_next, xt_next, 1, 1, G * 512)))
                            E(PAD)
                        else:
                            E(PAD)
                            E(lambda: emit_head(l, G, gst, xt_next))
                    return epi

                o_ps = ps_o.tile([P, QCW], f32, tag="o", name=f"o{l}{ci}")
                epi_fn = make_epi(o_ps=o_ps, og=og_tiles[G])
                for kt in range(KT):
                    ks0 = b * S + kt * P
                    # score QUAD (K=32: head duplicated -> 2x score)
                    scps = [ps_sc.tile([P, 2, QCW], f32, tag="sc",
                                       name=f"sc{l}{ci}{kt}{pr}")
                            for pr in range(2)]
                    for j in range(4):
                        nc.tensor.matmul(
                            scps[j // 2][:, j % 2, :],
                            slabs_cur["kt"][32 * j : 32 * j + 32, g,
                                            ks0 : ks0 + P],
                            slabs_cur["qt"][32 * j : 32 * j + 32, g,
                                            qs0 : qs0 + QCW],
                            start=True, stop=True,
                            tile_position=(32 * j, 0),
                        )
                    cur_e = []
                    for pr in range(2):
                        e_sb = epool.tile([P, 2, QCW], h16, tag="e",
                                          name=f"e{l}{ci}{kt}{pr}")
                        nc.scalar.activation(
                            out=e_sb.rearrange("p a q -> p (a q)"),
                            in_=scps[pr].rearrange("p a q -> p (a q)"),
                            func=AF.Exp, scale=float(SC / 2),
                        )
                        cur_e.append(e_sb)
                    if pending[0] is not None:
                        pending[0]()

                    def make_attnv(o_ps=o_ps, b=b, g=g, kt=kt,
                                   pe0=cur_e[0], pe1=cur_e[1],
                                   v=slabs_cur["v"],
                                   epi=(epi_fn if kt == KT - 1 else None)):
                        def attnv():
                            for j in range(4):
                                nc.tensor.matmul(
                                    o_ps[32 * j : 32 * j + 32, :],
                                    v[:, b * TPB + kt, 4 * g + j, :],
                                    (pe0 if j < 2 else pe1)[:, j % 2, :],
                                    start=(kt == 0), stop=(kt == KT - 1),
                                    tile_position=(0, 32 * j),
                                    skip_group_check=True,
                                )
                            if epi is not None:
                                epi()
                        return attnv

                    pending[0] = make_attnv()
                    pump()

            slabs_cur = slabs_next

        pending[0]()
        pending[0] = None
        while fillers:
            fillers.popleft()()
        ctx.close()

    nc.compile()
    return nc


def _get_nc():
    if "nc" not in _CACHE:
        _CACHE["nc"] = _build_nc()
    return _CACHE["nc"]


def kernel(**inputs) -> np.ndarray:
    from concourse.bass_utils import run_bass_kernel_spmd

    nc = _get_nc()
    ins = {k: np.ascontiguousarray(np.asarray(v)) for k, v in inputs.items()}
    in_maps = []
    for c in range(NCORES):
        m = dict(ins)
        m["x"] = np.ascontiguousarray(ins["x"][c * B_LOC : (c + 1) * B_LOC])
        in_maps.append(m)
    res = run_bass_kernel_spmd(nc, in_maps, list(range(NCORES)))
    out = np.concatenate([res.results[c]["out"] for c in range(NCORES)], axis=0)
    return out


# revision 25
# speedup vs baseline: 1.4479x; 1.0015x over previous
"""Trainium2 Bass kernel for nn_AttentionModel_87462714015827.

3-layer transformer encoder: B=16, S=1024, D=128, H=8 heads (DH=16),
FFN hidden 512, final 6-class projection.

Sharding: data-parallel over batch across 8 NeuronCores (2 batches/core),
all parameters replicated, no collectives.

v2 architecture:
  - ScalarE (ACT) does NOTHING but softmax exp; it is the kernel's
    metronome (~1.11us per 2-PSUM-bank exp, ~143us/layer).
  - All matmul streams are fp16 (f32 PSUM accumulate; fp16's 10-bit
    mantissa keeps the end-to-end error ~8x below bf16). Weights are
    DMA'd f32 and cast on-chip in the prologue.
  - NO PE transposes: every layout flip (x^T, x1^T, o, f) is a hardware
    XBAR DMA transpose issued on the otherwise-idle Sync engine.
  - PSUM: score pool 3 bufs x 2 banks (the 4 score matmuls of a kt
    issue back-to-back and run 4-way row-tiled concurrent), o 1 bank,
    mp 1 bank for all projection/FFN matmuls.
  - The LN/FFN/projection pipeline for each 512-token group is emitted
    as 16 small closures ("fillers"), popped one per kt iteration of
    the attention loop (64 slots = 4 groups x 16 per layer), so that
    work flows inside the PE/DVE slack under the continuous exp stream.
  - Scores use K=32 (each head duplicated in the slab) -> 2x score,
    folded into the exp scale.
"""

import os
import sys
from collections import deque

import numpy as np

for _p in ("/opt/trn_rl_repo", "/root/.axon_site/_ro/trn_rl_repo"):
    if os.path.isdir(_p) and _p not in sys.path:
        sys.path.insert(0, _p)

B, S, D, H, L = 16, 1024, 128, 8, 3
DFF = 4 * D          # 512
DH = D // H          # 16
NCLS = 6
NCORES = 8
B_LOC = B // NCORES  # 2
TOK = B_LOC * S      # 2048
TT = TOK // 128      # 16 token tiles per core
TPB = S // 128       # 8 token tiles per batch
P = 128
NQUAD = 2            # head quads (4 heads each)
KT = TPB             # 8 k tiles of 128 per batch
QCW = 512            # q-chunk width
NSUB = QCW // P      # 4 token tiles per group
NG = 4               # token groups (b, qc) per layer

_CACHE = {}


def _build_nc():
    import concourse.bass as bass
    import concourse.mybir as mybir
    import concourse.tile as tile
    from concourse import bacc

    dt = mybir.dt
    f32 = dt.float32
    h16 = dt.float16
    i32 = dt.int32
    AF = mybir.ActivationFunctionType
    OP = mybir.AluOpType

    nc = bacc.Bacc("TRN2", target_bir_lowering=False)

    # ---- DRAM I/O ----
    x_d = nc.dram_tensor("x", [B_LOC, S, D], f32, kind="ExternalInput")
    wq_d = nc.dram_tensor("Wq", [L, D, D], f32, kind="ExternalInput")
    bq_d = nc.dram_tensor("bq", [L, D], f32, kind="ExternalInput")
    wk_d = nc.dram_tensor("Wk", [L, D, D], f32, kind="ExternalInput")
    bk_d = nc.dram_tensor("bk", [L, D], f32, kind="ExternalInput")
    wv_d = nc.dram_tensor("Wv", [L, D, D], f32, kind="ExternalInput")
    bv_d = nc.dram_tensor("bv", [L, D], f32, kind="ExternalInput")
    l1g_d = nc.dram_tensor("ln1_g", [L, D], f32, kind="ExternalInput")
    l1b_d = nc.dram_tensor("ln1_b", [L, D], f32, kind="ExternalInput")
    w1_d = nc.dram_tensor("W1", [L, D, DFF], f32, kind="ExternalInput")
    b1_d = nc.dram_tensor("b1", [L, DFF], f32, kind="ExternalInput")
    w2_d = nc.dram_tensor("W2", [L, DFF, D], f32, kind="ExternalInput")
    b2_d = nc.dram_tensor("b2", [L, D], f32, kind="ExternalInput")
    l2g_d = nc.dram_tensor("ln2_g", [L, D], f32, kind="ExternalInput")
    l2b_d = nc.dram_tensor("ln2_b", [L, D], f32, kind="ExternalInput")
    wout_d = nc.dram_tensor("Wout", [D, NCLS], f32, kind="ExternalInput")
    bout_d = nc.dram_tensor("bout", [NCLS], f32, kind="ExternalInput")
    out_d = nc.dram_tensor("out", [B_LOC, S, NCLS], f32, kind="ExternalOutput")

    with tile.TileContext(nc) as tc:
        from contextlib import ExitStack

        ctx = ExitStack()
        cpool = ctx.enter_context(tc.tile_pool(name="const", bufs=1))
        apool = ctx.enter_context(tc.tile_pool(name="acts", bufs=2))
        gpool = ctx.enter_context(tc.tile_pool(name="grp", bufs=2))
        xpool = ctx.enter_context(tc.tile_pool(name="xprev", bufs=6))
        epool = ctx.enter_context(tc.tile_pool(name="epool", bufs=6))
        small = ctx.enter_context(tc.tile_pool(name="small", bufs=2))
        ps_sc = ctx.enter_context(tc.tile_pool(name="ps_sc", bufs=3, space="PSUM"))
        ps_o = ctx.enter_context(tc.tile_pool(name="ps_o", bufs=1, space="PSUM"))
        ps_mp = ctx.enter_context(tc.tile_pool(name="ps_mp", bufs=1, space="PSUM"))

        # HAM warmup first: no DMA deps, opens the PE clock gate at t=0.
        wu16 = cpool.tile([P, 512], h16)
        nc.vector.memset(wu16, 1.0)
        wup = ps_mp.tile([P, 512], f32, tag="mp", name="wup")
        for w in range(8):
            nc.tensor.matmul(wup, wu16[:, 0:P], wu16, start=True, stop=True)

        # ================= prologue: loads =================
        # x goes on the ACT HWDGE queue so it doesn't wait behind the
        # weight staging loads on the Sync queue (ACT is idle here).
        x_sb = cpool.tile([P, TT, D], f32)
        nc.sync.dma_start(out=x_sb, in_=x_d.rearrange("b (t p) d -> p (b t) d", p=P))

        wq_st = cpool.tile([P, L, D], f32)
        nc.sync.dma_start(out=wq_st, in_=wq_d.rearrange("l d e -> d l e"))
        wk_st = cpool.tile([P, L, D], f32)
        nc.sync.dma_start(out=wk_st, in_=wk_d.rearrange("l d e -> d l e"))
        wv_st = cpool.tile([P, L, D], f32)
        nc.sync.dma_start(out=wv_st, in_=wv_d.rearrange("l d e -> d l e"))
        w1_st = cpool.tile([P, L, DFF], f32)
        nc.sync.dma_start(out=w1_st, in_=w1_d.rearrange("l d f -> d l f"))
        w2_st = cpool.tile([P, L, 4, D], f32)
        nc.sync.dma_start(out=w2_st, in_=w2_d.rearrange("l (c p) e -> p l c e", p=P))
        b1c_sb = cpool.tile([P, L, 4], f32)
        nc.sync.dma_start(out=b1c_sb, in_=b1_d.rearrange("l (c p) -> p l c", p=P))
        b2_col = cpool.tile([P, L], f32)
        nc.sync.dma_start(out=b2_col, in_=b2_d.rearrange("l d -> d l"))
        wout_st = cpool.tile([P, NCLS], f32)
        nc.sync.dma_start(out=wout_st, in_=wout_d[:, :])

        # Q/K bias slabs in slab partition order via partition-scatter DMA:
        # partition 32j+16u+dh holds b[l, 64g+16j+dh] (u duplicates).
        def bias_slab(src_d, nm):
            t = cpool.tile([P, L, NQUAD], f32, name=f"bslab{nm}")
            for g in range(NQUAD):
                for u in range(2):
                    src = bass.AP(
                        tensor=src_d[0, 0].tensor, offset=64 * g,
                        ap=[[16, 4], [1, DH], [D, L]],
                    )
                    nc.gpsimd.dma_start(
                        out=t[:, :, g].rearrange("(j u e) l -> j u e l",
                                                 j=4, u=2)[:, u, :, :],
                        in_=src,
                    )
            return t

        bq_sb = bias_slab(bq_d, "q")
        bk_sb = bias_slab(bk_d, "k")

        # partition-replicated per-feature vectors
        _repn = [0]

        def rep_load(src_ap, shape):
            _repn[0] += 1
            t = cpool.tile([P] + shape, f32, name=f"rep{_repn[0]}")
            bc = bass.AP(tensor=src_ap.tensor, offset=src_ap.offset,
                         ap=[[0, P]] + [list(e) for e in src_ap.ap])
            nc.gpsimd.dma_start(out=t, in_=bc)
            return t

        bv_rep = rep_load(bv_d[:, :], [L, D])
        l1b_rep = rep_load(l1b_d[:, :], [L, D])
        l1g_rep = rep_load(l1g_d[:, :], [L, D])
        l2g_rep = rep_load(l2g_d[:, :], [L, D])
        l2b_rep = rep_load(l2b_d[:, :], [L, D])
        bout_rep = rep_load(bout_d[:], [NCLS])

        # ================= prologue: on-chip casts to fp16 =================
        # Q/K weight slabs: quad g, head 4g+j at cols 32j..32j+15 AND
        # duplicated at 32j+16..31 (K=32 scores read both copies).
        wq_sb = cpool.tile([P, L, NQUAD, P], h16)
        wk_sb = cpool.tile([P, L, NQUAD, P], h16)
        for (w_st, w_sb) in ((wq_st, wq_sb), (wk_st, wk_sb)):
            for l in range(L):
                for g in range(NQUAD):
                    src = (w_st[:, l, 64 * g : 64 * g + 64]
                           .rearrange("d (j e) -> d j e", j=4)
                           [:, :, None, :].to_broadcast([P, 4, 2, DH]))
                    nc.vector.tensor_copy(
                        w_sb[:, l, g, :].rearrange("p (j u e) -> p j u e",
                                                   j=4, u=2),
                        src,
                    )
        wv_sb = cpool.tile([P, L, D], h16)
        nc.vector.tensor_copy(wv_sb, wv_st)
        w1_sb = cpool.tile([P, L, DFF], h16)
        nc.vector.tensor_copy(w1_sb, w1_st)
        w2_sb = cpool.tile([P, L, 4, D], h16)
        nc.vector.tensor_copy(w2_sb, w2_st)
        wout_sb = cpool.tile([P, NCLS], h16)
        nc.vector.tensor_copy(wout_sb, wout_st)

        # x in fp16, then x^T via one batched DMA transpose (ACT queue)
        x16 = cpool.tile([P, TT, D], h16)
        nc.vector.tensor_copy(x16, x_sb)

        SC = 1.0 / np.sqrt(np.float32(DH))

        # ---------- helpers ----------
        def dma_T(out3, in3):
            """Batched XBAR transpose on the Sync HWDGE (fp16):
            out[do, di, m] = in[m, di, do]; in [M,Di,Do], out [Do,Di,M]."""
            nc.sync.dma_start(out=out3, in_=in3, transpose=True)

        def rsqrt_dve(rstd, var_ap, eps, w, tagp):
            """rstd = 1/sqrt(var+eps) on DVE (magic seed + 3 Newton steps)."""
            ve = small.tile([P, w], f32, tag="ve", name=f"ve{tagp}")
            nc.vector.tensor_scalar(out=ve, in0=var_ap, scalar1=float(eps),
                                    scalar2=None, op0=OP.add)
            yi = rstd.bitcast(i32)
            nc.vector.tensor_scalar(out=yi, in0=ve.bitcast(i32), scalar1=1,
                                    scalar2=None, op0=OP.logical_shift_right)
            nc.vector.tensor_scalar(out=yi, in0=yi, scalar1=0x5F3759DF,
                                    scalar2=-1, op0=OP.subtract, op1=OP.mult)
            nt = small.tile([P, w], f32, tag="nt", name=f"nt{tagp}")
            for _ in range(3):
                nc.vector.tensor_tensor(nt, rstd, rstd, OP.mult)
                nc.vector.tensor_tensor(nt, nt, ve, OP.mult)
                nc.vector.tensor_scalar(out=nt, in0=nt, scalar1=-0.5,
                                        scalar2=1.5, op0=OP.mult, op1=OP.add)
                nc.vector.tensor_tensor(rstd, rstd, nt, OP.mult)

        def alloc_layer_slabs(l):
            sl = {}
            sl["qt"] = apool.tile([P, NQUAD, TOK], h16, tag="qt", name=f"qt{l}")
            sl["kt"] = apool.tile([P, NQUAD, TOK], h16, tag="kt", name=f"kt{l}")
            sl["v"] = apool.tile([P, TT, H, 32], h16, tag="v", name=f"v{l}")
            nc.vector.memset(sl["v"][:, :, :, DH], 1.0)
            nc.vector.memset(sl["v"][:, :, :, DH + 1 : 32], 0.0)
            return sl

        def emit_qk_proj(l, sl, xt, proj, g, gq0):
            """One 512-token QK projection matmul + relu drain."""
            w_sb, b_sb, dston = (
                (wq_sb, bq_sb, "qt") if proj == 0 else (wk_sb, bk_sb, "kt"))
            pp = ps_mp.tile([P, 512], f32, tag="mp", name=f"pj{l}{proj}{g}{gq0}")
            nc.tensor.matmul(pp, w_sb[:, l, g, :], xt[:, gq0 : gq0 + 512],
                             start=True, stop=True)
            nc.vector.tensor_scalar(
                out=sl[dston][:, g, gq0 : gq0 + 512], in0=pp,
                scalar1=b_sb[:, l, g : g + 1], scalar2=0.0,
                op0=OP.add, op1=OP.max,
            )

        def emit_v_proj(l, sl, xt, t):
            """V projection for one 128-token tile + bias/relu drain."""
            pv = ps_mp.tile([P, D], f32, tag="mp", name=f"pv{l}{t}")
            nc.tensor.matmul(pv, xt[:, t * P : (t + 1) * P], wv_sb[:, l, :],
                             start=True, stop=True)
            nc.vector.tensor_tensor(
                sl["v"][:, t, :, 0:DH],
                pv.rearrange("p (h e) -> p h e", h=H),
                bv_rep[:, l, :].rearrange("p (h e) -> p h e", h=H),
                OP.add,
            )
            nc.vector.tensor_scalar(
                out=sl["v"][:, t, :, 0:DH], in0=sl["v"][:, t, :, 0:DH],
                scalar1=0.0, scalar2=None, op0=OP.max,
            )

        # ================= group pipeline stage pieces =================
        # Token group G=(b,qc): token tiles t0..t0+3, t0 = b*TPB + qc*NSUB.

        def ln_stats(src_tiles, mv, tagp):
            for i in range(NSUB):
                st6 = small.tile([P, 6], f32, tag="st6", name=f"st{tagp}{i}")
                nc.vector.bn_stats(out=st6, in_=src_tiles[:, i, :])
                nc.vector.bn_aggr(out=mv[:, i, :], in_=st6)

        def ln_norm(src, mv, rstd, dst):
            for i in range(NSUB):
                nc.vector.tensor_scalar(
                    out=dst[:, i, :], in0=src[:, i, :],
                    scalar1=mv[:, i, 0:1], scalar2=rstd[:, i : i + 1],
                    op0=OP.subtract, op1=OP.mult,
                )

        def s1_res(l, G, gst):
            res = gpool.tile([P, NSUB, D], f32, tag="res", name=f"res{l}{G}")
            mv = small.tile([P, NSUB, 2], f32, tag="mv", name=f"mv1{l}{G}")
            og, xprev_src = gst["og"], gst["xsrc"]
            b, qc = divmod(G, 2)
            t0 = b * TPB + qc * NSUB
            for i in range(NSUB):
                nc.vector.tensor_tensor(
                    res[:, i, :], og[:, i, :], xprev_src(t0 + i), OP.add)
            ln_stats(res, mv, f"1{l}{G}")
            gst.update(res=res, mv1=mv)

        def s1_ln(l, G, gst):
            rstd = small.tile([P, NSUB], f32, tag="rstd", name=f"rs1{l}{G}")
            rsqrt_dve(rstd, gst["mv1"][:, :, 1], 1e-8, NSUB, f"a{l}{G}")
            xn = gpool.tile([P, NSUB, D], f32, tag="xn", name=f"xn{l}{G}")
            ln_norm(gst["res"], gst["mv1"], rstd, xn)
            gst["xn"] = xn

        def s1_t1(l, G, gst, half):
            """t1 = xn*g1+b1 (fp16, gpsimd) for 2 tiles + their x1^T DMA."""
            if half == 0:
                gst["t1"] = gpool.tile([P, NSUB, D], h16, tag="t1",
                                       name=f"t1{l}{G}")
                gst["x1t"] = gpool.tile([P, NSUB, P], h16, tag="x1t",
                                        name=f"x1t{l}{G}")
            t1, xn = gst["t1"], gst["xn"]
            for i in (2 * half, 2 * half + 1):
                nc.gpsimd.tensor_tensor(
                    t1[:, i, :], xn[:, i, :], l1g_rep[:, l, :], OP.mult)
                nc.gpsimd.tensor_tensor(
                    t1[:, i, :], t1[:, i, :], l1b_rep[:, l, :], OP.add)
            dma_T(gst["x1t"][:, 2 * half : 2 * half + 2, :],
                  t1[:, 2 * half : 2 * half + 2, :])

        def s2_ffn1(l, G, gst, half):
            """FFN1 blocks 2h..2h+1: matmul + relu drain into ht."""
            if half == 0:
                gst["ht"] = gpool.tile([P, 4, 512], h16, tag="ht",
                                       name=f"ht{l}{G}")
            x1t = gst["x1t"].rearrange("d s m -> d (s m)")
            for c in (2 * half, 2 * half + 1):
                pp = ps_mp.tile([P, 512], f32, tag="mp", name=f"ph{l}{G}{c}")
                nc.tensor.matmul(pp, w1_sb[:, l, c * P : (c + 1) * P], x1t,
                                 start=True, stop=True)
                nc.vector.tensor_scalar(
                    out=gst["ht"][:, c, :], in0=pp,
                    scalar1=b1c_sb[:, l, c : c + 1], scalar2=0.0,
                    op0=OP.add, op1=OP.max,
                )

        def s2_ffn2(l, G, gst):
            """FFN2: 4 accumulating matmuls + b2 drain (fp16) + f^T DMA."""
            pf = ps_mp.tile([P, 512], f32, tag="mp", name=f"pf{l}{G}")
            for c in range(4):
                nc.tensor.matmul(pf, w2_sb[:, l, c, :], gst["ht"][:, c, :],
                                 start=(c == 0), stop=(c == 3))
            ft16 = gpool.tile([P, NSUB, P], h16, tag="ft16", name=f"ft{l}{G}")
            nc.vector.tensor_scalar(
                out=ft16.rearrange("d s m -> d (s m)"), in0=pf,
                scalar1=b2_col[:, l : l + 1], scalar2=None, op0=OP.add)
            ftt = gpool.tile([P, NSUB, P], h16, tag="ftt", name=f"ftt{l}{G}")
            dma_T(ftt, ft16)
            gst["ftt"] = ftt

        def s3_res2(l, G, gst):
            res2 = gpool.tile([P, NSUB, D], f32, tag="res2", name=f"re2{l}{G}")
            mv = small.tile([P, NSUB, 2], f32, tag="mv", name=f"mv2{l}{G}")
            for i in range(NSUB):
                nc.vector.tensor_tensor(
                    res2[:, i, :], gst["ftt"][:, i, :], gst["t1"][:, i, :],
                    OP.add)
            ln_stats(res2, mv, f"2{l}{G}")
            gst.update(res2=res2, mv2=mv)

        def s3_ln(l, G, gst):
            rstd = small.tile([P, NSUB], f32, tag="rstd", name=f"rs2{l}{G}")
            rsqrt_dve(rstd, gst["mv2"][:, :, 1], 1e-6, NSUB, f"b{l}{G}")
            xn2 = gpool.tile([P, NSUB, D], f32, tag="xn2", name=f"xn2{l}{G}")
            ln_norm(gst["res2"], gst["mv2"], rstd, xn2)
            gst["xn2"] = xn2

        def s3_xp(l, G, gst, xprev_next, xt_next, half):
            """xprev(l+1) = xn2*g2+b2 (fp16, gpsimd) + x^T(l+1) DMA."""
            xn2 = gst["xn2"]
            gq0 = G * 512
            for i in (2 * half, 2 * half + 1):
                nc.gpsimd.tensor_tensor(
                    xprev_next[:, i, :], xn2[:, i, :], l2g_rep[:, l, :],
                    OP.mult)
                nc.gpsimd.tensor_tensor(
                    xprev_next[:, i, :], xprev_next[:, i, :], l2b_rep[:, l, :],
                    OP.add)
            dma_T(xt_next[:, gq0 + 2 * half * P : gq0 + (2 * half + 2) * P]
                  .rearrange("d (t m) -> d t m", m=P),
                  xprev_next[:, 2 * half : 2 * half + 2, :])

        def emit_head(l, G, gst, xt):
            """Final 6-class projection for group G (4 tiny matmuls)."""
            b, qc = divmod(G, 2)
            t0 = b * TPB + qc * NSUB
            o6 = gpool.tile([P, NSUB, NCLS], f32, tag="o6", name=f"o6{G}")
            for i in range(NSUB):
                p6 = ps_mp.tile([P, NCLS], f32, tag="mp", name=f"p6{G}{i}")
                nc.tensor.matmul(p6, xt[:, (t0 + i) * P : (t0 + i + 1) * P],
                                 wout_sb, start=True, stop=True)
                nc.vector.tensor_tensor(o6[:, i, :], p6, bout_rep, OP.add)
            nc.sync.dma_start(
                out=out_d.rearrange("b (t p) c -> p (b t) c", p=P)
                [:, t0 : t0 + NSUB, :],
                in_=o6,
            )

        # ================= layer-0 x^T + projections (serial prologue) ====
        # x^T transpose on the ACT HWDGE queue (no waiting behind weights);
        # projections via the (still free) score pool, double-buffered.
        xt0 = apool.tile([P, TOK], h16, tag="xt", name="xt0")
        nc.sync.dma_start(out=xt0.rearrange("d (t m) -> d t m", m=P),
                          in_=x16, transpose=True)
        slabs0 = alloc_layer_slabs(0)
        for g in range(NQUAD):
            for proj in range(2):
                w_sb, b_sb, dston = (
                    (wq_sb, bq_sb, "qt") if proj == 0 else (wk_sb, bk_sb, "kt"))
                for cp in range(2):
                    sct = ps_sc.tile([P, 2, QCW], f32, tag="sc",
                                     name=f"p0{proj}{g}{cp}")
                    for h in range(2):
                        nc.tensor.matmul(
                            sct[:, h, :], w_sb[:, 0, g, :],
                            xt0[:, cp * 1024 + h * 512 : cp * 1024 + (h + 1) * 512],
                            start=True, stop=True)
                    nc.vector.tensor_scalar(
                        out=slabs0[dston][:, g, cp * 1024 : (cp + 1) * 1024],
                        in0=sct.rearrange("p a q -> p (a q)"),
                        scalar1=b_sb[:, 0, g : g + 1], scalar2=0.0,
                        op0=OP.add, op1=OP.max,
                    )
        for tp in range(TT // 2):
            sct = ps_sc.tile([P, 2, QCW], f32, tag="sc", name=f"pv0{tp}")
            for h in range(2):
                t = 2 * tp + h
                nc.tensor.matmul(sct[:, h, 0:D],
                                 xt0[:, t * P : (t + 1) * P], wv_sb[:, 0, :],
                                 start=True, stop=True)
            for h in range(2):
                t = 2 * tp + h
                nc.vector.tensor_tensor(
                    slabs0["v"][:, t, :, 0:DH],
                    sct[:, h, 0:D].rearrange("p (h2 e) -> p h2 e", h2=H),
                    bv_rep[:, 0, :].rearrange("p (h2 e) -> p h2 e", h2=H),
                    OP.add,
                )
                nc.vector.tensor_scalar(
                    out=slabs0["v"][:, t, :, 0:DH],
                    in0=slabs0["v"][:, t, :, 0:DH],
                    scalar1=0.0, scalar2=None, op0=OP.max,
                )

        # ================= main pipelined layer loop =================
        fillers = deque()

        def pump():
            n = 2 if len(fillers) > 18 else 1
            for _ in range(min(n, len(fillers))):
                fillers.popleft()()

        PAD = lambda: None
        slabs_cur = slabs0
        xprev_tiles = {}
        gstate = {}
        pending = [None]  # lagged attnv(+epilogue) of the previous kt step

        for l in range(L):
            last = l == L - 1
            if not last:
                slabs_next = alloc_layer_slabs(l + 1)
                xt_next = apool.tile([P, TOK], h16, tag="xt", name=f"xt{l+1}")
            else:
                slabs_next = None
                # final layer: xt_next holds (LN2*g2+b2)^T, the head input
                xt_next = apool.tile([P, TOK], h16, tag="xt", name="xtF")

            og_tiles = {}
            for ci, (b, qc, g) in enumerate(
                    [(b, qc, g) for b in range(B_LOC) for qc in range(2)
                     for g in range(NQUAD)]):
                G = b * 2 + qc
                qs0 = b * S + qc * QCW
                if g == 0:
                    og_tiles[G] = gpool.tile([P, NSUB, D], f32, tag="og",
                                             name=f"og{l}{G}")

                def make_epi(l=l, ci=ci, b=b, qc=qc, g=g, G=G,
                             o_ps=None, og=None, slabs_next=slabs_next,
                             xt_next=xt_next, last=last):
                    def epi():
                        ot16 = small.tile([P, QCW], h16, tag="ot16",
                                          name=f"ot{l}{ci}")
                        nc.vector.tensor_copy(ot16, o_ps)
                        ott = gpool.tile([P, NSUB, P], h16, tag="ott",
                                         name=f"ott{l}{ci}")
                        dma_T(ott, ot16.rearrange("p (s m) -> p s m", m=P))
                        rcp = small.tile([P, NSUB, 4], f32, tag="rcp",
                                         name=f"rcp{l}{ci}")
                        nc.vector.reciprocal(rcp, ott[:, :, DH :: 32])
                        nc.vector.tensor_tensor(
                            og[:, :, 64 * g : 64 * g + 64]
                                .rearrange("p s (j e) -> p s j e", j=4),
                            ott.rearrange("p s (j u) -> p s j u", j=4)
                            [:, :, :, 0:DH],
                            rcp[:, :, :, None].to_broadcast([P, NSUB, 4, DH]),
                            OP.mult,
                        )
                        if g != 1:
                            return
                        # enqueue this group's 16-pop F/P1 pipeline
                        gst = gstate.setdefault((l, G), {})
                        gst["og"] = og
                        if l == 0:
                            gst["xsrc"] = lambda t: x_sb[:, t, :]
                        else:
                            xp = xprev_tiles[(l, G)]
                            b_, qc_ = divmod(G, 2)
                            t0_ = b_ * TPB + qc_ * NSUB
                            gst["xsrc"] = (
                                lambda t, xp=xp, t0_=t0_: xp[:, t - t0_, :])
                        E = fillers.append
                        E(lambda: s1_res(l, G, gst))
                        E(lambda: s1_ln(l, G, gst))
                        E(lambda: s1_t1(l, G, gst, 0))
                        E(lambda: s1_t1(l, G, gst, 1))
                        E(PAD)
                        E(lambda: s2_ffn1(l, G, gst, 0))
                        E(lambda: s2_ffn1(l, G, gst, 1))
                        E(lambda: s2_ffn2(l, G, gst))
                        E(lambda: (s3_res2(l, G, gst), s3_ln(l, G, gst)))
                        xp_next = xpool.tile([P, NSUB, D], h16, tag="xp",
                                             name=f"xp{l+1}{G}")
                        if not last:
                            xprev_tiles[(l + 1, G)] = xp_next
                        E(lambda: s3_xp(l, G, gst, xp_next, xt_next, 0))
                        E(lambda: s3_xp(l, G, gst, xp_next, xt_next, 1))
                        if not last:
                            b_, qc_ = divmod(G, 2)
                            t0 = b_ * TPB + qc_ * NSUB
                            E(lambda: (emit_v_proj(l + 1, slabs_next, xt_next, t0),
                                       emit_v_proj(l + 1, slabs_next, xt_next, t0 + 1)))
                            E(lambda: (emit_v_proj(l + 1, slabs_next, xt_next, t0 + 2),
                                       emit_v_proj(l + 1, slabs_next, xt_next, t0 + 3)))
                            E(lambda: (emit_qk_proj(l + 1, slabs_next, xt_next, 0, 0, G * 512),
                                       emit_qk_proj(l + 1, slabs_next, xt_next, 1, 0, G * 512)))
                            E(lambda: (emit_qk_proj(l + 1, slabs_next, xt_next, 0, 1, G * 512),
                                       emit_qk_proj(l + 1, slabs_next, xt_next, 1, 1, G * 512)))
                            E(PAD)
                        else:
                            E(PAD)
                            E(lambda: emit_head(l, G, gst, xt_next))
                    return epi

                o_ps = ps_o.tile([P, QCW], f32, tag="o", name=f"o{l}{ci}")
                epi_fn = make_epi(o_ps=o_ps, og=og_tiles[G])
                for kt in range(KT):
                    ks0 = b * S + kt * P
                    # score QUAD (K=32: head duplicated -> 2x score)
                    scps = [ps_sc.tile([P, 2, QCW], f32, tag="sc",
                                       name=f"sc{l}{ci}{kt}{pr}")
                            for pr in range(2)]
                    for j in range(4):
                        nc.tensor.matmul(
                            scps[j // 2][:, j % 2, :],
                            slabs_cur["kt"][32 * j : 32 * j + 32, g,
                                            ks0 : ks0 + P],
                            slabs_cur["qt"][32 * j : 32 * j + 32, g,
                                            qs0 : qs0 + QCW],
                            start=True, stop=True,
                            tile_position=(32 * j, 0),
                        )
                    cur_e = []
                    for pr in range(2):
                        e_sb = epool.tile([P, 2, QCW], h16, tag="e",
                                          name=f"e{l}{ci}{kt}{pr}")
                        nc.scalar.activation(
                            out=e_sb.rearrange("p a q -> p (a q)"),
                            in_=scps[pr].rearrange("p a q -> p (a q)"),
                            func=AF.Exp, scale=float(SC / 2),
                        )
                        cur_e.append(e_sb)
                    if pending[0] is not None:
                        pending[0]()

                    def make_attnv(o_ps=o_ps, b=b, g=g, kt=kt,
                                   pe0=cur_e[0], pe1=cur_e[1],
                                   v=slabs_cur["v"],
                                   epi=(epi_fn if kt == KT - 1 else None)):
                        def attnv():
                            for j in range(4):
                                nc.tensor.matmul(
                                    o_ps[32 * j : 32 * j + 32, :],
                                    v[:, b * TPB + kt, 4 * g + j, :],
                                    (pe0 if j < 2 else pe1)[:, j % 2, :],
                                    start=(kt == 0), stop=(kt == KT - 1),
                                    tile_position=(0, 32 * j),
                                    skip_group_check=True,
                                )
                            if epi is not None:
                                epi()
                        return attnv

                    pending[0] = make_attnv()
                    pump()

            slabs_cur = slabs_next

        pending[0]()
        pending[0] = None
        while fillers:
            fillers.popleft()()
        ctx.close()

    nc.compile()
    return nc


def _get_nc():
    if "nc" not in _CACHE:
        _CACHE["nc"] = _build_nc()
    return _CACHE["nc"]


def kernel(**inputs) -> np.ndarray:
    from concourse.bass_utils import run_bass_kernel_spmd

    nc = _get_nc()
    ins = {k: np.ascontiguousarray(np.asarray(v)) for k, v in inputs.items()}
    in_maps = []
    for c in range(NCORES):
        m = dict(ins)
        m["x"] = np.ascontiguousarray(ins["x"][c * B_LOC : (c + 1) * B_LOC])
        in_maps.append(m)
    res = run_bass_kernel_spmd(nc, in_maps, list(range(NCORES)))
    out = np.concatenate([res.results[c]["out"] for c in range(NCORES)], axis=0)
    return out


# revision 30
# speedup vs baseline: 1.4482x; 1.0002x over previous
"""Trainium2 Bass kernel for nn_AttentionModel_87462714015827.

3-layer transformer encoder: B=16, S=1024, D=128, H=8 heads (DH=16),
FFN hidden 512, final 6-class projection.

Sharding: data-parallel over batch across 8 NeuronCores (2 batches/core),
all parameters replicated, no collectives.

v2 architecture:
  - ScalarE (ACT) does NOTHING but softmax exp; it is the kernel's
    metronome (~1.11us per 2-PSUM-bank exp, ~143us/layer).
  - All matmul streams are fp16 (f32 PSUM accumulate; fp16's 10-bit
    mantissa keeps the end-to-end error ~8x below bf16). Weights are
    DMA'd f32 and cast on-chip in the prologue.
  - NO PE transposes: every layout flip (x^T, x1^T, o, f) is a hardware
    XBAR DMA transpose issued on the otherwise-idle Sync engine.
  - PSUM: score pool 3 bufs x 2 banks (the 4 score matmuls of a kt
    issue back-to-back and run 4-way row-tiled concurrent), o 1 bank,
    mp 1 bank for all projection/FFN matmuls.
  - The LN/FFN/projection pipeline for each 512-token group is emitted
    as 16 small closures ("fillers"), popped one per kt iteration of
    the attention loop (64 slots = 4 groups x 16 per layer), so that
    work flows inside the PE/DVE slack under the continuous exp stream.
  - Scores use K=32 (each head duplicated in the slab) -> 2x score,
    folded into the exp scale.
"""

import os
import sys
from collections import deque

import numpy as np

for _p in ("/opt/trn_rl_repo", "/root/.axon_site/_ro/trn_rl_repo"):
    if os.path.isdir(_p) and _p not in sys.path:
        sys.path.insert(0, _p)

B, S, D, H, L = 16, 1024, 128, 8, 3
DFF = 4 * D          # 512
DH = D // H          # 16
NCLS = 6
NCORES = 8
B_LOC = B // NCORES  # 2
TOK = B_LOC * S      # 2048
TT = TOK // 128      # 16 token tiles per core
TPB = S // 128       # 8 token tiles per batch
P = 128
NQUAD = 2            # head quads (4 heads each)
KT = TPB             # 8 k tiles of 128 per batch
QCW = 512            # q-chunk width
NSUB = QCW // P      # 4 token tiles per group
NG = 4               # token groups (b, qc) per layer

_CACHE = {}


def _build_nc():
    import concourse.bass as bass
    import concourse.mybir as mybir
    import concourse.tile as tile
    from concourse import bacc

    dt = mybir.dt
    f32 = dt.float32
    h16 = dt.float16
    i32 = dt.int32
    AF = mybir.ActivationFunctionType
    OP = mybir.AluOpType

    nc = bacc.Bacc("TRN2", target_bir_lowering=False)

    # ---- DRAM I/O ----
    x_d = nc.dram_tensor("x", [B_LOC, S, D], f32, kind="ExternalInput")
    wq_d = nc.dram_tensor("Wq", [L, D, D], f32, kind="ExternalInput")
    bq_d = nc.dram_tensor("bq", [L, D], f32, kind="ExternalInput")
    wk_d = nc.dram_tensor("Wk", [L, D, D], f32, kind="ExternalInput")
    bk_d = nc.dram_tensor("bk", [L, D], f32, kind="ExternalInput")
    wv_d = nc.dram_tensor("Wv", [L, D, D], f32, kind="ExternalInput")
    bv_d = nc.dram_tensor("bv", [L, D], f32, kind="ExternalInput")
    l1g_d = nc.dram_tensor("ln1_g", [L, D], f32, kind="ExternalInput")
    l1b_d = nc.dram_tensor("ln1_b", [L, D], f32, kind="ExternalInput")
    w1_d = nc.dram_tensor("W1", [L, D, DFF], f32, kind="ExternalInput")
    b1_d = nc.dram_tensor("b1", [L, DFF], f32, kind="ExternalInput")
    w2_d = nc.dram_tensor("W2", [L, DFF, D], f32, kind="ExternalInput")
    b2_d = nc.dram_tensor("b2", [L, D], f32, kind="ExternalInput")
    l2g_d = nc.dram_tensor("ln2_g", [L, D], f32, kind="ExternalInput")
    l2b_d = nc.dram_tensor("ln2_b", [L, D], f32, kind="ExternalInput")
    wout_d = nc.dram_tensor("Wout", [D, NCLS], f32, kind="ExternalInput")
    bout_d = nc.dram_tensor("bout", [NCLS], f32, kind="ExternalInput")
    out_d = nc.dram_tensor("out", [B_LOC, S, NCLS], f32, kind="ExternalOutput")

    with tile.TileContext(nc) as tc:
        from contextlib import ExitStack

        ctx = ExitStack()
        cpool = ctx.enter_context(tc.tile_pool(name="const", bufs=1))
        apool = ctx.enter_context(tc.tile_pool(name="acts", bufs=2))
        gpool = ctx.enter_context(tc.tile_pool(name="grp", bufs=2))
        xpool = ctx.enter_context(tc.tile_pool(name="xprev", bufs=6))
        epool = ctx.enter_context(tc.tile_pool(name="epool", bufs=6))
        small = ctx.enter_context(tc.tile_pool(name="small", bufs=2))
        ps_sc = ctx.enter_context(tc.tile_pool(name="ps_sc", bufs=3, space="PSUM"))
        ps_o = ctx.enter_context(tc.tile_pool(name="ps_o", bufs=1, space="PSUM"))
        ps_mp = ctx.enter_context(tc.tile_pool(name="ps_mp", bufs=1, space="PSUM"))

        # HAM warmup first: no DMA deps, opens the PE clock gate at t=0.
        wu16 = cpool.tile([P, 512], h16)
        nc.vector.memset(wu16, 1.0)
        wup = ps_mp.tile([P, 512], f32, tag="mp", name="wup")
        for w in range(8):
            nc.tensor.matmul(wup, wu16[:, 0:P], wu16, start=True, stop=True)

        # ================= prologue: loads =================
        # x goes on the ACT HWDGE queue so it doesn't wait behind the
        # weight staging loads on the Sync queue (ACT is idle here).
        x_sb = cpool.tile([P, TT, D], f32)
        nc.scalar.dma_start(out=x_sb, in_=x_d.rearrange("b (t p) d -> p (b t) d", p=P))

        wq_st = cpool.tile([P, L, D], f32)
        nc.sync.dma_start(out=wq_st, in_=wq_d.rearrange("l d e -> d l e"))
        wk_st = cpool.tile([P, L, D], f32)
        nc.sync.dma_start(out=wk_st, in_=wk_d.rearrange("l d e -> d l e"))
        wv_st = cpool.tile([P, L, D], f32)
        nc.sync.dma_start(out=wv_st, in_=wv_d.rearrange("l d e -> d l e"))
        w1_st = cpool.tile([P, L, DFF], f32)
        nc.sync.dma_start(out=w1_st, in_=w1_d.rearrange("l d f -> d l f"))
        w2_st = cpool.tile([P, L, 4, D], f32)
        nc.sync.dma_start(out=w2_st, in_=w2_d.rearrange("l (c p) e -> p l c e", p=P))
        b1c_sb = cpool.tile([P, L, 4], f32)
        nc.sync.dma_start(out=b1c_sb, in_=b1_d.rearrange("l (c p) -> p l c", p=P))
        b2_col = cpool.tile([P, L], f32)
        nc.sync.dma_start(out=b2_col, in_=b2_d.rearrange("l d -> d l"))
        wout_st = cpool.tile([P, NCLS], f32)
        nc.sync.dma_start(out=wout_st, in_=wout_d[:, :])

        # Q/K bias slabs in slab partition order via partition-scatter DMA:
        # partition 32j+16u+dh holds b[l, 64g+16j+dh] (u duplicates).
        def bias_slab(src_d, nm):
            t = cpool.tile([P, L, NQUAD], f32, name=f"bslab{nm}")
            for g in range(NQUAD):
                for j in range(4):
                    for u in range(2):
                        src = bass.AP(
                            tensor=src_d[0, 0].tensor,
                            offset=64 * g + 16 * j,
                            ap=[[1, DH], [D, L]],
                        )
                        nc.gpsimd.dma_start(
                            out=t[32 * j + 16 * u : 32 * j + 16 * u + DH,
                                  :, g],
                            in_=src,
                        )
            return t

        bq_sb = bias_slab(bq_d, "q")
        bk_sb = bias_slab(bk_d, "k")

        # partition-replicated per-feature vectors
        _repn = [0]

        def rep_load(src_ap, shape):
            _repn[0] += 1
            t = cpool.tile([P] + shape, f32, name=f"rep{_repn[0]}")
            bc = bass.AP(tensor=src_ap.tensor, offset=src_ap.offset,
                         ap=[[0, P]] + [list(e) for e in src_ap.ap])
            nc.gpsimd.dma_start(out=t, in_=bc)
            return t

        bv_rep = rep_load(bv_d[:, :], [L, D])
        l1b_rep = rep_load(l1b_d[:, :], [L, D])
        l1g_rep = rep_load(l1g_d[:, :], [L, D])
        l2g_rep = rep_load(l2g_d[:, :], [L, D])
        l2b_rep = rep_load(l2b_d[:, :], [L, D])
        bout_rep = rep_load(bout_d[:], [NCLS])

        # ================= prologue: on-chip casts to fp16 =================
        # Q/K weight slabs: quad g, head 4g+j at cols 32j..32j+15 AND
        # duplicated at 32j+16..31 (K=32 scores read both copies).
        wq_sb = cpool.tile([P, L, NQUAD, P], h16)
        wk_sb = cpool.tile([P, L, NQUAD, P], h16)
        for (w_st, w_sb) in ((wq_st, wq_sb), (wk_st, wk_sb)):
            for l in range(L):
                for g in range(NQUAD):
                    src = (w_st[:, l, 64 * g : 64 * g + 64]
                           .rearrange("d (j e) -> d j e", j=4)
                           [:, :, None, :].to_broadcast([P, 4, 2, DH]))
                    nc.vector.tensor_copy(
                        w_sb[:, l, g, :].rearrange("p (j u e) -> p j u e",
                                                   j=4, u=2),
                        src,
                    )
        wv_sb = cpool.tile([P, L, D], h16)
        nc.vector.tensor_copy(wv_sb, wv_st)
        w1_sb = cpool.tile([P, L, DFF], h16)
        nc.vector.tensor_copy(w1_sb, w1_st)
        w2_sb = cpool.tile([P, L, 4, D], h16)
        nc.vector.tensor_copy(w2_sb, w2_st)
        wout_sb = cpool.tile([P, NCLS], h16)
        nc.vector.tensor_copy(wout_sb, wout_st)

        # x in fp16, then x^T via one batched DMA transpose (ACT queue)
        x16 = cpool.tile([P, TT, D], h16)
        nc.vector.tensor_copy(x16, x_sb)

        SC = 1.0 / np.sqrt(np.float32(DH))

        # ---------- helpers ----------
        def dma_T(out3, in3):
            """Batched XBAR transpose on the Sync HWDGE (fp16):
            out[do, di, m] = in[m, di, do]; in [M,Di,Do], out [Do,Di,M]."""
            nc.sync.dma_start(out=out3, in_=in3, transpose=True)

        def rsqrt_dve(rstd, var_ap, eps, w, tagp):
            """rstd = 1/sqrt(var+eps) on DVE (magic seed + 3 Newton steps)."""
            ve = small.tile([P, w], f32, tag="ve", name=f"ve{tagp}")
            nc.vector.tensor_scalar(out=ve, in0=var_ap, scalar1=float(eps),
                                    scalar2=None, op0=OP.add)
            yi = rstd.bitcast(i32)
            nc.vector.tensor_scalar(out=yi, in0=ve.bitcast(i32), scalar1=1,
                                    scalar2=None, op0=OP.logical_shift_right)
            nc.vector.tensor_scalar(out=yi, in0=yi, scalar1=0x5F3759DF,
                                    scalar2=-1, op0=OP.subtract, op1=OP.mult)
            nt = small.tile([P, w], f32, tag="nt", name=f"nt{tagp}")
            for _ in range(3):
                nc.vector.tensor_tensor(nt, rstd, rstd, OP.mult)
                nc.vector.tensor_tensor(nt, nt, ve, OP.mult)
                nc.vector.tensor_scalar(out=nt, in0=nt, scalar1=-0.5,
                                        scalar2=1.5, op0=OP.mult, op1=OP.add)
                nc.vector.tensor_tensor(rstd, rstd, nt, OP.mult)

        def alloc_layer_slabs(l):
            sl = {}
            sl["qt"] = apool.tile([P, NQUAD, TOK], h16, tag="qt", name=f"qt{l}")
            sl["kt"] = apool.tile([P, NQUAD, TOK], h16, tag="kt", name=f"kt{l}")
            sl["v"] = apool.tile([P, TT, H, 32], h16, tag="v", name=f"v{l}")
            nc.vector.memset(sl["v"][:, :, :, DH], 1.0)
            nc.vector.memset(sl["v"][:, :, :, DH + 1 : 32], 0.0)
            return sl

        def emit_qk_proj(l, sl, xt, proj, g, gq0):
            """One 512-token QK projection matmul + relu drain."""
            w_sb, b_sb, dston = (
                (wq_sb, bq_sb, "qt") if proj == 0 else (wk_sb, bk_sb, "kt"))
            pp = ps_mp.tile([P, 512], f32, tag="mp", name=f"pj{l}{proj}{g}{gq0}")
            nc.tensor.matmul(pp, w_sb[:, l, g, :], xt[:, gq0 : gq0 + 512],
                             start=True, stop=True)
            nc.vector.tensor_scalar(
                out=sl[dston][:, g, gq0 : gq0 + 512], in0=pp,
                scalar1=b_sb[:, l, g : g + 1], scalar2=0.0,
                op0=OP.add, op1=OP.max,
            )

        def emit_v_proj(l, sl, xt, t):
            """V projection for one 128-token tile + bias/relu drain."""
            pv = ps_mp.tile([P, D], f32, tag="mp", name=f"pv{l}{t}")
            nc.tensor.matmul(pv, xt[:, t * P : (t + 1) * P], wv_sb[:, l, :],
                             start=True, stop=True)
            nc.vector.tensor_tensor(
                sl["v"][:, t, :, 0:DH],
                pv.rearrange("p (h e) -> p h e", h=H),
                bv_rep[:, l, :].rearrange("p (h e) -> p h e", h=H),
                OP.add,
            )
            nc.vector.tensor_scalar(
                out=sl["v"][:, t, :, 0:DH], in0=sl["v"][:, t, :, 0:DH],
                scalar1=0.0, scalar2=None, op0=OP.max,
            )

        # ================= group pipeline stage pieces =================
        # Token group G=(b,qc): token tiles t0..t0+3, t0 = b*TPB + qc*NSUB.

        def ln_stats(src_tiles, mv, tagp):
            for i in range(NSUB):
                st6 = small.tile([P, 6], f32, tag="st6", name=f"st{tagp}{i}")
                nc.vector.bn_stats(out=st6, in_=src_tiles[:, i, :])
                nc.vector.bn_aggr(out=mv[:, i, :], in_=st6)

        def ln_norm(src, mv, rstd, dst):
            for i in range(NSUB):
                nc.vector.tensor_scalar(
                    out=dst[:, i, :], in0=src[:, i, :],
                    scalar1=mv[:, i, 0:1], scalar2=rstd[:, i : i + 1],
                    op0=OP.subtract, op1=OP.mult,
                )

        def s1_res(l, G, gst):
            res = gpool.tile([P, NSUB, D], f32, tag="res", name=f"res{l}{G}")
            mv = small.tile([P, NSUB, 2], f32, tag="mv", name=f"mv1{l}{G}")
            og, xprev_src = gst["og"], gst["xsrc"]
            b, qc = divmod(G, 2)
            t0 = b * TPB + qc * NSUB
            for i in range(NSUB):
                nc.vector.tensor_tensor(
                    res[:, i, :], og[:, i, :], xprev_src(t0 + i), OP.add)
            ln_stats(res, mv, f"1{l}{G}")
            gst.update(res=res, mv1=mv)

        def s1_ln(l, G, gst):
            rstd = small.tile([P, NSUB], f32, tag="rstd", name=f"rs1{l}{G}")
            rsqrt_dve(rstd, gst["mv1"][:, :, 1], 1e-8, NSUB, f"a{l}{G}")
            xn = gpool.tile([P, NSUB, D], f32, tag="xn", name=f"xn{l}{G}")
            ln_norm(gst["res"], gst["mv1"], rstd, xn)
            gst["xn"] = xn

        def s1_t1(l, G, gst, half):
            """t1 = xn*g1+b1 (fp16, gpsimd) for 2 tiles + their x1^T DMA."""
            if half == 0:
                gst["t1"] = gpool.tile([P, NSUB, D], h16, tag="t1",
                                       name=f"t1{l}{G}")
                gst["x1t"] = gpool.tile([P, NSUB, P], h16, tag="x1t",
                                        name=f"x1t{l}{G}")
            t1, xn = gst["t1"], gst["xn"]
            for i in (2 * half, 2 * half + 1):
                nc.gpsimd.tensor_tensor(
                    t1[:, i, :], xn[:, i, :], l1g_rep[:, l, :], OP.mult)
                nc.gpsimd.tensor_tensor(
                    t1[:, i, :], t1[:, i, :], l1b_rep[:, l, :], OP.add)
            dma_T(gst["x1t"][:, 2 * half : 2 * half + 2, :],
                  t1[:, 2 * half : 2 * half + 2, :])

        def s2_ffn1(l, G, gst, half):
            """FFN1 blocks 2h..2h+1: matmul + relu drain into ht."""
            if half == 0:
                gst["ht"] = gpool.tile([P, 4, 512], h16, tag="ht",
                                       name=f"ht{l}{G}")
            x1t = gst["x1t"].rearrange("d s m -> d (s m)")
            for c in (2 * half, 2 * half + 1):
                pp = ps_mp.tile([P, 512], f32, tag="mp", name=f"ph{l}{G}{c}")
                nc.tensor.matmul(pp, w1_sb[:, l, c * P : (c + 1) * P], x1t,
                                 start=True, stop=True)
                nc.vector.tensor_scalar(
                    out=gst["ht"][:, c, :], in0=pp,
                    scalar1=b1c_sb[:, l, c : c + 1], scalar2=0.0,
                    op0=OP.add, op1=OP.max,
                )

        def s2_ffn2(l, G, gst):
            """FFN2: 4 accumulating matmuls + b2 drain (fp16) + f^T DMA."""
            pf = ps_mp.tile([P, 512], f32, tag="mp", name=f"pf{l}{G}")
            for c in range(4):
                nc.tensor.matmul(pf, w2_sb[:, l, c, :], gst["ht"][:, c, :],
                                 start=(c == 0), stop=(c == 3))
            ft16 = gpool.tile([P, NSUB, P], h16, tag="ft16", name=f"ft{l}{G}")
            nc.vector.tensor_scalar(
                out=ft16.rearrange("d s m -> d (s m)"), in0=pf,
                scalar1=b2_col[:, l : l + 1], scalar2=None, op0=OP.add)
            ftt = gpool.tile([P, NSUB, P], h16, tag="ftt", name=f"ftt{l}{G}")
            dma_T(ftt, ft16)
            gst["ftt"] = ftt

        def s3_res2(l, G, gst):
            res2 = gpool.tile([P, NSUB, D], f32, tag="res2", name=f"re2{l}{G}")
            mv = small.tile([P, NSUB, 2], f32, tag="mv", name=f"mv2{l}{G}")
            for i in range(NSUB):
                nc.vector.tensor_tensor(
                    res2[:, i, :], gst["ftt"][:, i, :], gst["t1"][:, i, :],
                    OP.add)
            ln_stats(res2, mv, f"2{l}{G}")
            gst.update(res2=res2, mv2=mv)

        def s3_ln(l, G, gst):
            rstd = small.tile([P, NSUB], f32, tag="rstd", name=f"rs2{l}{G}")
            rsqrt_dve(rstd, gst["mv2"][:, :, 1], 1e-6, NSUB, f"b{l}{G}")
            xn2 = gpool.tile([P, NSUB, D], f32, tag="xn2", name=f"xn2{l}{G}")
            ln_norm(gst["res2"], gst["mv2"], rstd, xn2)
            gst["xn2"] = xn2

        def s3_xp(l, G, gst, xprev_next, xt_next, half):
            """xprev(l+1) = xn2*g2+b2 (fp16, gpsimd) + x^T(l+1) DMA."""
            xn2 = gst["xn2"]
            gq0 = G * 512
            for i in (2 * half, 2 * half + 1):
                nc.gpsimd.tensor_tensor(
                    xprev_next[:, i, :], xn2[:, i, :], l2g_rep[:, l, :],
                    OP.mult)
                nc.gpsimd.tensor_tensor(
                    xprev_next[:, i, :], xprev_next[:, i, :], l2b_rep[:, l, :],
                    OP.add)
            dma_T(xt_next[:, gq0 + 2 * half * P : gq0 + (2 * half + 2) * P]
                  .rearrange("d (t m) -> d t m", m=P),
                  xprev_next[:, 2 * half : 2 * half + 2, :])

        def emit_head(l, G, gst, xt):
            """Final 6-class projection for group G (4 tiny matmuls)."""
            b, qc = divmod(G, 2)
            t0 = b * TPB + qc * NSUB
            o6 = gpool.tile([P, NSUB, NCLS], f32, tag="o6", name=f"o6{G}")
            for i in range(NSUB):
                p6 = ps_mp.tile([P, NCLS], f32, tag="mp", name=f"p6{G}{i}")
                nc.tensor.matmul(p6, xt[:, (t0 + i) * P : (t0 + i + 1) * P],
                                 wout_sb, start=True, stop=True)
                nc.vector.tensor_tensor(o6[:, i, :], p6, bout_rep, OP.add)
            nc.sync.dma_start(
                out=out_d.rearrange("b (t p) c -> p (b t) c", p=P)
                [:, t0 : t0 + NSUB, :],
                in_=o6,
            )

        # ================= layer-0 x^T + projections (serial prologue) ====
        # x^T transpose on the ACT HWDGE queue (no waiting behind weights);
        # projections via the (still free) score pool, double-buffered.
        xt0 = apool.tile([P, TOK], h16, tag="xt", name="xt0")
        nc.scalar.dma_start(out=xt0.rearrange("d (t m) -> d t m", m=P),
                            in_=x16, transpose=True)
        slabs0 = alloc_layer_slabs(0)
        for g in range(NQUAD):
            for proj in range(2):
                w_sb, b_sb, dston = (
                    (wq_sb, bq_sb, "qt") if proj == 0 else (wk_sb, bk_sb, "kt"))
                for cp in range(2):
                    sct = ps_sc.tile([P, 2, QCW], f32, tag="sc",
                                     name=f"p0{proj}{g}{cp}")
                    for h in range(2):
                        nc.tensor.matmul(
                            sct[:, h, :], w_sb[:, 0, g, :],
                            xt0[:, cp * 1024 + h * 512 : cp * 1024 + (h + 1) * 512],
                            start=True, stop=True)
                    nc.vector.tensor_scalar(
                        out=slabs0[dston][:, g, cp * 1024 : (cp + 1) * 1024],
                        in0=sct.rearrange("p a q -> p (a q)"),
                        scalar1=b_sb[:, 0, g : g + 1], scalar2=0.0,
                        op0=OP.add, op1=OP.max,
                    )
        for tp in range(TT // 2):
            sct = ps_sc.tile([P, 2, QCW], f32, tag="sc", name=f"pv0{tp}")
            for h in range(2):
                t = 2 * tp + h
                nc.tensor.matmul(sct[:, h, 0:D],
                                 xt0[:, t * P : (t + 1) * P], wv_sb[:, 0, :],
                                 start=True, stop=True)
            for h in range(2):
                t = 2 * tp + h
                nc.vector.tensor_tensor(
                    slabs0["v"][:, t, :, 0:DH],
                    sct[:, h, 0:D].rearrange("p (h2 e) -> p h2 e", h2=H),
                    bv_rep[:, 0, :].rearrange("p (h2 e) -> p h2 e", h2=H),
                    OP.add,
                )
                nc.vector.tensor_scalar(
                    out=slabs0["v"][:, t, :, 0:DH],
                    in0=slabs0["v"][:, t, :, 0:DH],
                    scalar1=0.0, scalar2=None, op0=OP.max,
                )

        # ================= main pipelined layer loop =================
        fillers = deque()

        def pump():
            n = 2 if len(fillers) > 18 else 1
            for _ in range(min(n, len(fillers))):
                fillers.popleft()()

        PAD = lambda: None
        slabs_cur = slabs0
        xprev_tiles = {}
        gstate = {}
        pending = [None]  # lagged attnv(+epilogue) of the previous kt step

        for l in range(L):
            last = l == L - 1
            if not last:
                slabs_next = alloc_layer_slabs(l + 1)
                xt_next = apool.tile([P, TOK], h16, tag="xt", name=f"xt{l+1}")
            else:
                slabs_next = None
                # final layer: xt_next holds (LN2*g2+b2)^T, the head input
                xt_next = apool.tile([P, TOK], h16, tag="xt", name="xtF")

            og_tiles = {}
            for ci, (b, qc, g) in enumerate(
                    [(b, qc, g) for b in range(B_LOC) for qc in range(2)
                     for g in range(NQUAD)]):
                G = b * 2 + qc
                qs0 = b * S + qc * QCW
                if g == 0:
                    og_tiles[G] = gpool.tile([P, NSUB, D], f32, tag="og",
                                             name=f"og{l}{G}")

                def make_epi(l=l, ci=ci, b=b, qc=qc, g=g, G=G,
                             o_ps=None, og=None, slabs_next=slabs_next,
                             xt_next=xt_next, last=last):
                    def epi():
                        ot16 = small.tile([P, QCW], h16, tag="ot16",
                                          name=f"ot{l}{ci}")
                        nc.vector.tensor_copy(ot16, o_ps)
                        ott = gpool.tile([P, NSUB, P], h16, tag="ott",
                                         name=f"ott{l}{ci}")
                        dma_T(ott, ot16.rearrange("p (s m) -> p s m", m=P))
                        rcp = small.tile([P, NSUB, 4], f32, tag="rcp",
                                         name=f"rcp{l}{ci}")
                        nc.vector.reciprocal(rcp, ott[:, :, DH :: 32])
                        nc.vector.tensor_tensor(
                            og[:, :, 64 * g : 64 * g + 64]
                                .rearrange("p s (j e) -> p s j e", j=4),
                            ott.rearrange("p s (j u) -> p s j u", j=4)
                            [:, :, :, 0:DH],
                            rcp[:, :, :, None].to_broadcast([P, NSUB, 4, DH]),
                            OP.mult,
                        )
                        if g != 1:
                            return
                        # enqueue this group's 16-pop F/P1 pipeline
                        gst = gstate.setdefault((l, G), {})
                        gst["og"] = og
                        if l == 0:
                            gst["xsrc"] = lambda t: x_sb[:, t, :]
                        else:
                            xp = xprev_tiles[(l, G)]
                            b_, qc_ = divmod(G, 2)
                            t0_ = b_ * TPB + qc_ * NSUB
                            gst["xsrc"] = (
                                lambda t, xp=xp, t0_=t0_: xp[:, t - t0_, :])
                        E = fillers.append
                        E(lambda: s1_res(l, G, gst))
                        E(lambda: s1_ln(l, G, gst))
                        E(lambda: s1_t1(l, G, gst, 0))
                        E(lambda: s1_t1(l, G, gst, 1))
                        E(PAD)
                        E(lambda: s2_ffn1(l, G, gst, 0))
                        E(lambda: s2_ffn1(l, G, gst, 1))
                        E(lambda: s2_ffn2(l, G, gst))
                        E(lambda: (s3_res2(l, G, gst), s3_ln(l, G, gst)))
                        xp_next = xpool.tile([P, NSUB, D], h16, tag="xp",
                                             name=f"xp{l+1}{G}")
                        if not last:
                            xprev_tiles[(l + 1, G)] = xp_next
                        E(lambda: s3_xp(l, G, gst, xp_next, xt_next, 0))
                        E(lambda: s3_xp(l, G, gst, xp_next, xt_next, 1))
                        if not last:
                            b_, qc_ = divmod(G, 2)
                            t0 = b_ * TPB + qc_ * NSUB
                            E(lambda: (emit_v_proj(l + 1, slabs_next, xt_next, t0),
                                       emit_v_proj(l + 1, slabs_next, xt_next, t0 + 1)))
                            E(lambda: (emit_v_proj(l + 1, slabs_next, xt_next, t0 + 2),
                                       emit_v_proj(l + 1, slabs_next, xt_next, t0 + 3)))
                            E(lambda: (emit_qk_proj(l + 1, slabs_next, xt_next, 0, 0, G * 512),
                                       emit_qk_proj(l + 1, slabs_next, xt_next, 1, 0, G * 512)))
                            E(lambda: (emit_qk_proj(l + 1, slabs_next, xt_next, 0, 1, G * 512),
                                       emit_qk_proj(l + 1, slabs_next, xt_next, 1, 1, G * 512)))
                            E(PAD)
                        else:
                            E(PAD)
                            E(lambda: emit_head(l, G, gst, xt_next))
                    return epi

                o_ps = ps_o.tile([P, QCW], f32, tag="o", name=f"o{l}{ci}")
                epi_fn = make_epi(o_ps=o_ps, og=og_tiles[G])
                for kt in range(KT):
                    ks0 = b * S + kt * P
                    # score QUAD (K=32: head duplicated -> 2x score)
                    scps = [ps_sc.tile([P, 2, QCW], f32, tag="sc",
                                       name=f"sc{l}{ci}{kt}{pr}")
                            for pr in range(2)]
                    for j in range(4):
                        nc.tensor.matmul(
                            scps[j // 2][:, j % 2, :],
                            slabs_cur["kt"][32 * j : 32 * j + 32, g,
                                            ks0 : ks0 + P],
                            slabs_cur["qt"][32 * j : 32 * j + 32, g,
                                            qs0 : qs0 + QCW],
                            start=True, stop=True,
                            tile_position=(32 * j, 0),
                        )
                    cur_e = []
                    for pr in range(2):
                        e_sb = epool.tile([P, 2, QCW], h16, tag="e",
                                          name=f"e{l}{ci}{kt}{pr}")
                        nc.scalar.activation(
                            out=e_sb.rearrange("p a q -> p (a q)"),
                            in_=scps[pr].rearrange("p a q -> p (a q)"),
                            func=AF.Exp, scale=float(SC / 2),
                        )
                        cur_e.append(e_sb)
                    if pending[0] is not None:
                        pending[0]()

                    def make_attnv(o_ps=o_ps, b=b, g=g, kt=kt,
                                   pe0=cur_e[0], pe1=cur_e[1],
                                   v=slabs_cur["v"],
                                   epi=(epi_fn if kt == KT - 1 else None)):
                        def attnv():
                            for j in range(4):
                                nc.tensor.matmul(
                                    o_ps[32 * j : 32 * j + 32, :],
                                    v[:, b * TPB + kt, 4 * g + j, :],
                                    (pe0 if j < 2 else pe1)[:, j % 2, :],
                                    start=(kt == 0), stop=(kt == KT - 1),
                                    tile_position=(0, 32 * j),
                                    skip_group_check=True,
                                )
                            if epi is not None:
                                epi()
                        return attnv

                    pending[0] = make_attnv()
                    pump()

            slabs_cur = slabs_next

        if pending[0] is not None:
            pending[0]()
            pending[0] = None
        while fillers:
            fillers.popleft()()
        ctx.close()

    nc.compile()
    return nc


def _get_nc():
    if "nc" not in _CACHE:
        _CACHE["nc"] = _build_nc()
    return _CACHE["nc"]


def kernel(**inputs) -> np.ndarray:
    from concourse.bass_utils import run_bass_kernel_spmd

    nc = _get_nc()
    ins = {k: np.ascontiguousarray(np.asarray(v)) for k, v in inputs.items()}
    in_maps = []
    for c in range(NCORES):
        m = dict(ins)
        m["x"] = np.ascontiguousarray(ins["x"][c * B_LOC : (c + 1) * B_LOC])
        in_maps.append(m)
    res = run_bass_kernel_spmd(nc, in_maps, list(range(NCORES)))
    out = np.concatenate([res.results[c]["out"] for c in range(NCORES)], axis=0)
    return out
